# revision 20
# baseline (speedup 1.0000x reference)
"""SoftRas-style soft rasterizer on 8 Trainium2 NeuronCores.

Strategy:
- All per-(face,pixel) affine quantities (barycentric w0/w1, edge projections
  u/l2, squared vertex distances) are produced by TensorE matmuls against the
  pixel basis [1, px, py, px^2+py^2] (K=4).
- The nonlinear chain (clip/sqrt/sigmoid/reciprocal/exp/log) runs on
  VectorE/ScalarE with faces on partitions (128/chunk), pixels on the free dim.
- Per-pixel max over faces (softmax shift) via PE transposes + free-dim max.
- Face-direction reductions (rgb accumulation, dsum, sum log(1-p)) via PE
  matmuls against textures / ones.
- Host (numpy): per-face coefficient prep, per-tile face culling, per-core
  load balancing (every core gets an identical chunk-count pattern so one
  SPMD program serves all 8 cores), final divide + alpha exponentiation.
"""
import sys
sys.path.insert(0, '/opt/trn_rl_repo')
import numpy as np
import ml_dtypes
from contextlib import ExitStack

import concourse.bass as bass
import concourse.bacc as bacc
import concourse.tile as tile
import concourse.mybir as mybir
from concourse.bass_utils import run_bass_kernel_spmd
from concourse.masks import make_identity

TRACE = False
LAST_RESULT = None

F_TOT = 512
H = W = 256
NCORES = 8
TP = 512              # pixels per tile position (2 image rows)
NPOS = (H * W) // (NCORES * TP)   # 16 tile positions per core
SIGMA = 1e-2
GAMMA = 1e-3
EPS = 1e-3
NEAR, FAR = 1.0, 100.0
FP = mybir.dt.float32
F32R = mybir.dt.float32r
BF = mybir.dt.bfloat16
AL = mybir.AluOpType
AF = mybir.ActivationFunctionType


def _host_prep(face_vertices):
    """Per-face coefficients in basis [1, px, py, r2], fp64 -> fp32."""
    fv = np.asarray(face_vertices, np.float64)[0]          # [F,3,3]
    F = fv.shape[0]
    x = fv[:, :, 0]; y = fv[:, :, 1]; z = fv[:, :, 2]
    x0, x1, x2 = x[:, 0], x[:, 1], x[:, 2]
    y0, y1, y2 = y[:, 0], y[:, 1], y[:, 2]

    den = (y1 - y2) * (x0 - x2) + (x2 - x1) * (y0 - y2)
    den = np.where(np.abs(den) < 1e-10, 1e-10, den)
    W0c = np.stack([(-(y1 - y2) * x2 - (x2 - x1) * y2) / den,
                    (y1 - y2) / den, (x2 - x1) / den, np.zeros(F)], -1)
    W1c = np.stack([(-(y2 - y0) * x2 - (x0 - x2) * y2) / den,
                    (y2 - y0) / den, (x0 - x2) / den, np.zeros(F)], -1)

    anchors = [(x0, y0), (x1, y1), (x2, y2)]
    pairs = [(0, 1), (1, 2), (2, 0)]
    # per edge: U = ((p-a).d)/|d| (along-line coord), LD = cross(p-a, d)/|d|
    # (signed line distance). d2seg = LD^2 + max(|U - L/2| - L/2, 0)^2.
    UT = np.zeros((3, F, 4)); S2 = np.zeros((3, F, 4)); HL = np.zeros((3, F))
    for e, (ia, ib) in enumerate(pairs):
        ax, ay = anchors[ia]; bx, by = anchors[ib]
        dx, dy = bx - ax, by - ay
        L = np.sqrt(np.maximum(dx * dx + dy * dy, 1e-12))
        iL = 1.0 / L
        UT[e, :, 0] = (-ax * dx - ay * dy) * iL
        UT[e, :, 1] = dx * iL
        UT[e, :, 2] = dy * iL
        S2[e, :, 0] = (ay * dx - ax * dy) * iL
        S2[e, :, 1] = dy * iL
        S2[e, :, 2] = -dx * iL
        HL[e] = L / 2.0
    iz = 1.0 / z
    zmin = z.min(1); zmax = z.max(1)
    assert z.min() > NEAR + 0.05 and z.max() < FAR - 0.05, \
        "kernel fast path assumes all vertex depths strictly inside (NEAR,FAR)"
    return dict(W0c=W0c, W1c=W1c, UT=UT, S2=S2, HL=HL, iz=iz,
                ymin=y.min(1), ymax=y.max(1), zmin=zmin, zmax=zmax)


def _cull_and_balance(prep):
    """Per 2-row band, the kept-face list; balanced so all cores share one
    chunk-count pattern. Returns (pattern, assign) where
    assign[core][pos] = (band_index, face_index_array padded with -1)."""
    nb = H // 2
    pixy = ((np.arange(H) + 0.5) / H) * 2.0 - 1.0
    ylo = pixy[0::2]; yhi = pixy[1::2]                     # [nb]
    gap = np.maximum(0.0, np.maximum(prep['ymin'][None, :] - yhi[:, None],
                                     ylo[:, None] - prep['ymax'][None, :]))
    znUB = (FAR - prep['zmin']) / (FAR - NEAR)
    znLB = (FAR - prep['zmax']) / (FAR - NEAR)
    D = znLB.max()
    keep = (gap < 0.17) | ((gap / SIGMA) + (D - znUB) / GAMMA < 87.0)  # [nb,F]

    counts = np.maximum(1, np.ceil(keep.sum(1) / 128).astype(int))
    order = np.argsort(-counts, kind='stable')             # bands, desc count
    pattern = [int(counts[order[p * NCORES]]) for p in range(NPOS)]
    assign = [[None] * NPOS for _ in range(NCORES)]
    for p in range(NPOS):
        for c in range(NCORES):
            b = int(order[p * NCORES + c])
            faces = np.nonzero(keep[b])[0]
            pad = pattern[p] * 128 - len(faces)
            assert pad >= 0
            faces = np.concatenate([faces, -np.ones(pad, np.int64)])
            assign[c][p] = (b, faces)
    return pattern, assign


# 3-way bf16 split: x = h + m + l with each part bf16-exact. Products of
# bf16-exact values are exact in the PE's f32r mode, so a 6-combo expansion
# (dropping <1e-7 cross terms) gives fp32-class precision at full PE rate.
COMBOS = [(0, 0), (0, 1), (1, 0), (0, 2), (1, 1), (2, 0)]
NK = 4 * len(COMBOS)


def _split3(a):
    a = np.asarray(a, np.float64)
    h = a.astype(ml_dtypes.bfloat16).astype(np.float64)
    r = a - h
    m = r.astype(ml_dtypes.bfloat16).astype(np.float64)
    l = (r - m).astype(ml_dtypes.bfloat16).astype(np.float64)
    return [h, m, l]


def _face_arrays(prep, textures, faces):
    """Pack per-chunk coefficient/texture/scalar arrays for one chunk of 128
    face slots (index -1 = inert dummy)."""
    f = np.asarray(faces)
    dummy = f < 0
    fi = np.where(dummy, 0, f)

    def D(a):  # zero out dummies
        a = np.asarray(a, np.float64).copy()
        a[dummy] = 0.0
        return a

    # quantity order: U01,LD01,U12,LD12,U20,LD20,W0,W1,W2 -> coef[4, 9, 128]
    coef = np.zeros((4, 9, 128))
    for e in range(3):
        coef[:, 2 * e, :] = D(prep['UT'][e][fi]).T
        coef[:, 2 * e + 1, :] = D(prep['S2'][e][fi]).T
    coef[:, 6, :] = D(prep['W0c'][fi]).T
    coef[:, 7, :] = D(prep['W1c'][fi]).T
    # dummies: W0=W1=-1 (outside, wc2=1), LD=10 (dist 10 -> prob 0),
    # iz=0.011 -> zp~90.9 -> zn~0.092 (never the argmax), halfL=0.5
    coef[0, 1, dummy] = 10.0
    coef[0, 3, dummy] = 10.0
    coef[0, 5, dummy] = 10.0
    coef[0, 6, dummy] = -1.0
    coef[0, 7, dummy] = -1.0
    coef[:, 8, :] = -coef[:, 6, :] - coef[:, 7, :]
    coef[0, 8, :] += 1.0                                   # w2 = 1 - w0 - w1
    cs = _split3(coef)
    coefk = np.zeros((NK, 9, 128), np.float32)
    for t, (ci, bi) in enumerate(COMBOS):
        coefk[4 * t:4 * t + 4] = cs[ci].astype(np.float32)

    tex = np.asarray(textures, np.float64)[0][fi]          # [128,3,3] (k,c)
    tex[dummy] = 0.0

    scal = np.zeros((128, 9))
    izf = prep['iz'][fi]
    izf[dummy] = 0.011
    scal[:, 0:3] = izf
    hlf = prep['HL'][:, fi].T
    hlf[dummy] = 0.5
    scal[:, 3:6] = hlf
    scal[:, 6:9] = -hlf
    return coefk, tex, scal


def _build_program(pattern):
    """One SPMD Bass program; chunk counts per position given by pattern."""
    totc = sum(pattern)
    kmax = max(pattern)
    nc = bacc.Bacc("TRN2", target_bir_lowering=False, debug=False,
                   num_devices=NCORES)
    d_coef = nc.dram_tensor("coef", [totc, 24, 9 * 128], F32R, kind="ExternalInput")
    d_basis = nc.dram_tensor("basis", [NPOS, 24, TP], F32R, kind="ExternalInput")
    d_tex = nc.dram_tensor("tex", [128, totc * 9], FP, kind="ExternalInput")
    d_scal = nc.dram_tensor("scal", [128, totc * 9], FP, kind="ExternalInput")
    d_out = nc.dram_tensor("out6", [6, NPOS * TP], FP, kind="ExternalOutput")

    with ExitStack() as ctx:
        tc = ctx.enter_context(tile.TileContext(nc))
        const = ctx.enter_context(tc.tile_pool(name="const", bufs=1))
        stage = ctx.enter_context(tc.tile_pool(name="stage", bufs=2))
        basp = ctx.enter_context(tc.tile_pool(name="basp", bufs=2))
        work = ctx.enter_context(tc.tile_pool(name="work", bufs=1))
        store = ctx.enter_context(tc.tile_pool(name="store", bufs=2))
        zm = ctx.enter_context(tc.tile_pool(name="zm", bufs=2))
        qp = ctx.enter_context(tc.tile_pool(name="qp", bufs=6, space="PSUM"))
        accp = ctx.enter_context(tc.tile_pool(name="accp", bufs=2, space="PSUM"))

        ident = const.tile([128, 128], FP)
        make_identity(nc, ident)
        onesc = const.tile([128, 1], BF)
        nc.vector.memset(onesc, 1.0)
        onesf = const.tile([128, 1], FP)
        nc.vector.memset(onesf, 1.0)
        b_sqrt = const.tile([128, 1], FP)
        nc.vector.memset(b_sqrt, 1e-12)
        b_ln = const.tile([128, 1], FP)
        nc.vector.memset(b_ln, 1e-30)
        tex_sb = const.tile([128, totc * 9], FP)
        nc.sync.dma_start(out=tex_sb, in_=d_tex[:, :])
        scal_sb = const.tile([128, totc * 9], FP)
        nc.sync.dma_start(out=scal_sb, in_=d_scal[:, :])

        jj = 0
        for pos in range(NPOS):
            K = pattern[pos]
            bas = basp.tile([24, TP], F32R, tag="bas")
            nc.sync.dma_start(out=bas, in_=d_basis[pos, :, :])

            zn_st = store.tile([128, kmax, TP], FP, tag="zn_st")
            pv_st = store.tile([128, kmax, TP], FP, tag="pv_st")
            wn_st = [store.tile([128, kmax, TP], FP, tag=f"wn{k}_st",
                                 name=f"wn{k}_st") for k in range(3)]
            d2_st = store.tile([128, kmax, TP], FP, tag="d2_st")
            sb_st = store.tile([128, kmax, TP], mybir.dt.uint32, tag="sb_st")
            acc = accp.tile([65, TP], FP, tag="acc")

            # ---- phase 1a: per chunk, everything except sqrt/sigmoid/ln ----
            for j in range(K):
                cj = jj + j
                st = stage.tile([24, 9 * 128], F32R, tag="st")
                nc.sync.dma_start(out=st, in_=d_coef[cj, :, :])
                q = [qp.tile([128, TP], FP, tag="q", name=f"q{qi}")
                     for qi in range(9)]
                for qi in range(9):
                    nc.tensor.matmul(q[qi], st[:, qi * 128:(qi + 1) * 128],
                                     bas, start=True, stop=True)
                sc = lambda i: scal_sb[:, cj * 9 + i: cj * 9 + i + 1]

                # d2_e = LD^2 + max(|U - L/2| - L/2, 0)^2
                ru = [work.tile([128, TP], FP, tag=f"ru{e}", name=f"ru{e}")
                      for e in range(3)]
                tt = [work.tile([128, TP], FP, tag=f"t{e}", name=f"t{e}")
                      for e in range(3)]
                for e in range(3):
                    nc.scalar.activation(ru[e], q[2 * e + 1], AF.Square)
                    nc.scalar.activation(tt[e], q[2 * e], AF.Identity,
                                         bias=sc(6 + e))        # U - L/2
                    nc.vector.tensor_scalar(
                        out=tt[e].bitcast(mybir.dt.uint32),
                        in0=tt[e].bitcast(mybir.dt.uint32),
                        scalar1=0x7fffffff, scalar2=None,
                        op0=AL.bitwise_and)                     # abs
                    nc.scalar.activation(tt[e], tt[e], AF.Relu,
                                         bias=sc(6 + e))        # overshoot
                    nc.scalar.activation(tt[e], tt[e], AF.Square)
                    nc.vector.tensor_tensor(out=ru[e], in0=ru[e], in1=tt[e],
                                            op=AL.add)
                nc.vector.tensor_tensor(out=ru[0], in0=ru[0], in1=ru[1],
                                        op=AL.min)
                nc.vector.tensor_tensor(out=d2_st[:, j, :], in0=ru[0],
                                        in1=ru[2], op=AL.min)

                cw0 = work.tile([128, TP], FP, tag="cw0")
                nc.scalar.activation(cw0, q[6], AF.Copy)
                cw1 = work.tile([128, TP], FP, tag="cw1")
                nc.scalar.activation(cw1, q[7], AF.Copy)
                cw2 = work.tile([128, TP], FP, tag="cw2")
                nc.scalar.activation(cw2, q[8], AF.Copy)
                m1 = work.tile([128, TP], FP, tag="m1")
                nc.vector.tensor_tensor(out=m1, in0=cw0, in1=cw1, op=AL.min)
                nc.vector.tensor_tensor(out=m1, in0=m1, in1=cw2, op=AL.min)
                nc.vector.tensor_scalar(out=sb_st[:, j, :],
                                        in0=m1.bitcast(mybir.dt.uint32),
                                        scalar1=0x80000000, scalar2=None,
                                        op0=AL.bitwise_and)

                wc0, wc1, wc2 = cw0, cw1, cw2
                nc.vector.tensor_scalar(out=wc0, in0=cw0, scalar1=0.0,
                                        scalar2=1.0, op0=AL.max, op1=AL.min)
                nc.vector.tensor_scalar(out=wc1, in0=cw1, scalar1=0.0,
                                        scalar2=1.0, op0=AL.max, op1=AL.min)
                nc.vector.tensor_scalar(out=wc2, in0=cw2, scalar1=0.0,
                                        scalar2=1.0, op0=AL.max, op1=AL.min)
                s01 = work.tile([128, TP], FP, tag="s01")
                nc.vector.tensor_tensor(out=s01, in0=wc0, in1=wc1, op=AL.add)
                nc.vector.tensor_tensor(out=s01, in0=s01, in1=wc2, op=AL.add)
                invs = work.tile([128, TP], FP, tag="invs")
                nc.vector.reciprocal_approx_fast(out=invs, in_=s01)
                r1 = work.tile([128, TP], FP, tag="r1")
                nc.vector.tensor_scalar(out=r1, in0=wc0, scalar1=sc(0),
                                        scalar2=None, op0=AL.mult)
                nc.vector.scalar_tensor_tensor(out=r1, in0=wc1, scalar=sc(1),
                                               in1=r1, op0=AL.mult, op1=AL.add)
                nc.vector.scalar_tensor_tensor(out=r1, in0=wc2, scalar=sc(2),
                                               in1=r1, op0=AL.mult, op1=AL.add)
                nc.vector.tensor_tensor(out=r1, in0=r1, in1=invs, op=AL.mult)
                nc.vector.reciprocal_approx_fast(out=s01, in_=r1)   # zp
                nc.vector.tensor_scalar(out=zn_st[:, j, :], in0=s01,
                                        scalar1=-1.0 / (FAR - NEAR),
                                        scalar2=FAR / (FAR - NEAR),
                                        op0=AL.mult, op1=AL.add)
                for k, wck in enumerate([wc0, wc1, wc2]):
                    nc.gpsimd.tensor_tensor(out=wn_st[k][:, j, :], in0=wck,
                                            in1=invs, op=AL.mult)

            # ---- phase 1b: batched sqrt / sign / sigmoid / ln ----
            for j in range(K):
                nc.scalar.activation(d2_st[:, j, :], d2_st[:, j, :], AF.Sqrt,
                                     bias=b_sqrt)
            for j in range(K):
                nc.vector.tensor_tensor(
                    out=d2_st[:, j, :].bitcast(mybir.dt.uint32),
                    in0=d2_st[:, j, :].bitcast(mybir.dt.uint32),
                    in1=sb_st[:, j, :], op=AL.bitwise_or)
            for j in range(K):
                nc.scalar.activation(pv_st[:, j, :], d2_st[:, j, :],
                                     AF.Sigmoid, scale=1.0 / SIGMA)
            for j in range(K):
                q1m = work.tile([128, TP], FP, tag="q1m")
                nc.vector.tensor_scalar(out=q1m, in0=pv_st[:, j, :],
                                        scalar1=-1.0, scalar2=1.0,
                                        op0=AL.mult, op1=AL.add)
                lq = work.tile([128, TP], FP, tag="lq")
                nc.scalar.activation(lq, q1m, AF.Ln, bias=b_ln)
                nc.tensor.matmul(acc[64:65, :], onesf[:, 0:1], lq,
                                 start=(j == 0), stop=(j == K - 1))

            # ---- phase 2: per-pixel zmax (negated) via transposes ----
            zmc = zm.tile([128, 4], FP, tag="zmc")
            dts = [zm.tile([128, kmax * 128], FP, tag=f"dt{s}", name=f"dt{s}")
                   for s in range(4)]
            for s in range(4):
                znt = qp.tile([128, kmax * 128], FP, tag="q")
                for j in range(K):
                    nc.tensor.transpose(
                        znt[:, j * 128:(j + 1) * 128],
                        zn_st[:, j, s * 128:(s + 1) * 128], ident)
                nc.vector.tensor_reduce(
                    out=zmc[:, s:s + 1], in_=znt[:, 0:K * 128],
                    axis=mybir.AxisListType.X, op=AL.max)
                nc.vector.tensor_scalar(out=zmc[:, s:s + 1],
                                        in0=zmc[:, s:s + 1], scalar1=EPS,
                                        scalar2=-1.0, op0=AL.max, op1=AL.mult)
                nc.scalar.activation(dts[s][:, 0:K * 128], znt[:, 0:K * 128],
                                     AF.Identity, bias=zmc[:, s:s + 1])

            # ---- phase 3: exp weights, rgb/dsum accumulation ----
            for j in range(K):
                cj = jj + j
                dp = qp.tile([128, TP], FP, tag="q")
                for s in range(4):
                    nc.tensor.transpose(dp[:, s * 128:(s + 1) * 128],
                                        dts[s][:, j * 128:(j + 1) * 128],
                                        ident)
                d = work.tile([128, TP], FP, tag="d")
                nc.scalar.activation(d, dp, AF.Exp, scale=1.0 / GAMMA)
                nc.gpsimd.tensor_tensor(out=d, in0=pv_st[:, j, :], in1=d,
                                        op=AL.mult)             # wexp
                for k in range(3):
                    g = work.tile([128, TP], FP, tag="g", bufs=3)
                    nc.gpsimd.tensor_tensor(out=g, in0=d,
                                            in1=wn_st[k][:, j, :], op=AL.mult)
                    nc.tensor.matmul(
                        acc[0:3, :],
                        tex_sb[:, cj * 9 + k * 3: cj * 9 + (k + 1) * 3], g,
                        start=(j == 0 and k == 0),
                        stop=(j == K - 1 and k == 2))
                nc.tensor.matmul(acc[32:33, :], onesf[:, 0:1], d,
                                 start=(j == 0), stop=(j == K - 1))

            o6 = zm.tile([65, TP], FP, tag="o6")
            nc.scalar.activation(o6[0:3, :], acc[0:3, :], AF.Copy)
            nc.scalar.activation(o6[32:33, :], acc[32:33, :], AF.Copy)
            nc.scalar.activation(o6[64:65, :], acc[64:65, :], AF.Copy)
            nc.sync.dma_start(out=d_out[0:3, pos * TP:(pos + 1) * TP],
                              in_=o6[0:3, :])
            nc.sync.dma_start(out=d_out[3:4, pos * TP:(pos + 1) * TP],
                              in_=o6[32:33, :])
            nc.sync.dma_start(out=d_out[4:5, pos * TP:(pos + 1) * TP],
                              in_=o6[64:65, :])
            nc.sync.dma_start(
                out=d_out[5, pos * TP:(pos + 1) * TP].rearrange(
                    "(s p) -> p s", s=4),
                in_=zmc)
            jj += K
    nc.compile()
    return nc


def kernel(face_vertices, face_textures):
    prep = _host_prep(face_vertices)
    pattern, assign = _cull_and_balance(prep)
    totc = sum(pattern)

    pix = ((np.arange(H, dtype=np.float64) + 0.5) / H) * 2.0 - 1.0
    px_row = np.tile(pix, 2)                                # within a band
    in_maps = []
    for c in range(NCORES):
        coef = np.zeros((totc, NK, 9 * 128), np.float32)
        tex = np.zeros((128, totc * 9), np.float32)
        scal = np.zeros((128, totc * 9), np.float32)
        basis = np.zeros((NPOS, NK, TP), np.float32)
        jj = 0
        for pos in range(NPOS):
            b, faces = assign[c][pos]
            py = np.repeat(pix[2 * b:2 * b + 2], W)
            b4 = np.stack([np.ones(TP), px_row, py, px_row ** 2 + py ** 2])
            bs = _split3(b4)
            for t, (ci, bi) in enumerate(COMBOS):
                basis[pos, 4 * t:4 * t + 4] = bs[bi].astype(np.float32)
            for j in range(pattern[pos]):
                cf, tx, sl = _face_arrays(prep, face_textures,
                                          faces[j * 128:(j + 1) * 128])
                coef[jj] = cf.reshape(NK, 9 * 128)
                tex[:, jj * 9:(jj + 1) * 9] = tx.reshape(128, 9)
                scal[:, jj * 9:(jj + 1) * 9] = sl
                jj += 1
        in_maps.append({"coef": coef, "basis": basis, "tex": tex, "scal": scal})

    nc = _build_program(pattern)
    global LAST_RESULT
    if TRACE:
        res = run_bass_kernel_spmd(nc, in_maps, core_ids=list(range(NCORES)),
                                   trace=True)
    else:
        res = run_bass_kernel_spmd(nc, in_maps, core_ids=list(range(NCORES)))
    LAST_RESULT = res

    out = np.zeros((1, 4, H, W), np.float32)
    for c in range(NCORES):
        o6 = res.results[c]["out6"]                        # [6, NPOS*TP]
        for pos in range(NPOS):
            b, _ = assign[c][pos]
            seg = o6[:, pos * TP:(pos + 1) * TP]
            zmax = -seg[5]
            wbg = np.exp((EPS - zmax) / GAMMA).astype(np.float32)
            dsum = seg[3] + wbg
            rgb = seg[0:3] / dsum[None]
            alpha = 1.0 - np.exp(seg[4])
            out[0, 0:3, 2 * b:2 * b + 2, :] = rgb.reshape(3, 2, W)
            out[0, 3, 2 * b:2 * b + 2, :] = alpha.reshape(2, W)
    return out


# revision 21
# speedup vs baseline: 1.3797x; 1.3797x over previous
"""SoftRas-style soft rasterizer on 8 Trainium2 NeuronCores.

Strategy:
- All per-(face,pixel) affine quantities (barycentric w0/w1, edge projections
  u/l2, squared vertex distances) are produced by TensorE matmuls against the
  pixel basis [1, px, py, px^2+py^2] (K=4).
- The nonlinear chain (clip/sqrt/sigmoid/reciprocal/exp/log) runs on
  VectorE/ScalarE with faces on partitions (128/chunk), pixels on the free dim.
- Per-pixel max over faces (softmax shift) via PE transposes + free-dim max.
- Face-direction reductions (rgb accumulation, dsum, sum log(1-p)) via PE
  matmuls against textures / ones.
- Host (numpy): per-face coefficient prep, per-tile face culling, per-core
  load balancing (every core gets an identical chunk-count pattern so one
  SPMD program serves all 8 cores), final divide + alpha exponentiation.
"""
import sys
sys.path.insert(0, '/opt/trn_rl_repo')
import numpy as np
import ml_dtypes
from contextlib import ExitStack

import concourse.bass as bass
import concourse.bacc as bacc
import concourse.tile as tile
import concourse.mybir as mybir
from concourse.bass_utils import run_bass_kernel_spmd
from concourse.masks import make_identity

TRACE = False
LAST_RESULT = None

F_TOT = 512
H = W = 256
NCORES = 8
TP = 512              # pixels per tile position (2 image rows)
NPOS = (H * W) // (NCORES * TP)   # 16 tile positions per core
SIGMA = 1e-2
GAMMA = 1e-3
EPS = 1e-3
NEAR, FAR = 1.0, 100.0
FP = mybir.dt.float32
F32R = mybir.dt.float32r
BF = mybir.dt.bfloat16
AL = mybir.AluOpType
AF = mybir.ActivationFunctionType


def _host_prep(face_vertices):
    """Per-face coefficients in basis [1, px, py, r2], fp64 -> fp32."""
    fv = np.asarray(face_vertices, np.float64)[0]          # [F,3,3]
    F = fv.shape[0]
    x = fv[:, :, 0]; y = fv[:, :, 1]; z = fv[:, :, 2]
    x0, x1, x2 = x[:, 0], x[:, 1], x[:, 2]
    y0, y1, y2 = y[:, 0], y[:, 1], y[:, 2]

    den = (y1 - y2) * (x0 - x2) + (x2 - x1) * (y0 - y2)
    den = np.where(np.abs(den) < 1e-10, 1e-10, den)
    W0c = np.stack([(-(y1 - y2) * x2 - (x2 - x1) * y2) / den,
                    (y1 - y2) / den, (x2 - x1) / den, np.zeros(F)], -1)
    W1c = np.stack([(-(y2 - y0) * x2 - (x0 - x2) * y2) / den,
                    (y2 - y0) / den, (x0 - x2) / den, np.zeros(F)], -1)

    anchors = [(x0, y0), (x1, y1), (x2, y2)]
    pairs = [(0, 1), (1, 2), (2, 0)]
    # per edge: U = ((p-a).d)/|d| (along-line coord), LD = cross(p-a, d)/|d|
    # (signed line distance). d2seg = LD^2 + max(|U - L/2| - L/2, 0)^2.
    UT = np.zeros((3, F, 4)); S2 = np.zeros((3, F, 4)); HL = np.zeros((3, F))
    for e, (ia, ib) in enumerate(pairs):
        ax, ay = anchors[ia]; bx, by = anchors[ib]
        dx, dy = bx - ax, by - ay
        L = np.sqrt(np.maximum(dx * dx + dy * dy, 1e-12))
        iL = 1.0 / L
        UT[e, :, 0] = (-ax * dx - ay * dy) * iL
        UT[e, :, 1] = dx * iL
        UT[e, :, 2] = dy * iL
        S2[e, :, 0] = (ay * dx - ax * dy) * iL
        S2[e, :, 1] = dy * iL
        S2[e, :, 2] = -dx * iL
        HL[e] = L / 2.0
    iz = 1.0 / z
    zmin = z.min(1); zmax = z.max(1)
    assert z.min() > NEAR + 0.05 and z.max() < FAR - 0.05, \
        "kernel fast path assumes all vertex depths strictly inside (NEAR,FAR)"
    return dict(W0c=W0c, W1c=W1c, UT=UT, S2=S2, HL=HL, iz=iz,
                ymin=y.min(1), ymax=y.max(1), xmin=x.min(1), xmax=x.max(1),
                zmin=zmin, zmax=zmax)


def _cull_and_balance(prep):
    """Per tile (4 rows x 128 px), the kept-face list; balanced so all cores
    share one chunk-count pattern. Returns (pattern, assign) where
    assign[core][pos] = (tile_index, face_index_array padded with -1)."""
    nyb = H // 4
    pixc = ((np.arange(H) + 0.5) / H) * 2.0 - 1.0
    tiles = []
    for yb in range(nyb):
        for xb in range(2):
            tiles.append((pixc[4 * yb], pixc[4 * yb + 3],
                          pixc[128 * xb], pixc[128 * xb + 127]))
    tiles = np.array(tiles)                                # [nb, 4]
    nb = len(tiles)
    ygap = np.maximum(0.0, np.maximum(
        prep['ymin'][None, :] - tiles[:, 1:2],
        tiles[:, 0:1] - prep['ymax'][None, :]))
    xgap = np.maximum(0.0, np.maximum(
        prep['xmin'][None, :] - tiles[:, 3:4],
        tiles[:, 2:3] - prep['xmax'][None, :]))
    gap = np.sqrt(xgap ** 2 + ygap ** 2)
    znUB = (FAR - prep['zmin']) / (FAR - NEAR)
    znLB = (FAR - prep['zmax']) / (FAR - NEAR)
    D = znLB.max()
    keep = (gap < 0.17) | ((gap / SIGMA) + (D - znUB) / GAMMA < 87.0)  # [nb,F]

    counts = np.maximum(1, np.ceil(keep.sum(1) / 128).astype(int))
    order = np.argsort(-counts, kind='stable')             # bands, desc count
    pattern = [int(counts[order[p * NCORES]]) for p in range(NPOS)]
    assign = [[None] * NPOS for _ in range(NCORES)]
    for p in range(NPOS):
        for c in range(NCORES):
            b = int(order[p * NCORES + c])
            faces = np.nonzero(keep[b])[0]
            pad = pattern[p] * 128 - len(faces)
            assert pad >= 0
            faces = np.concatenate([faces, -np.ones(pad, np.int64)])
            assign[c][p] = (b, faces)
    return pattern, assign


# 3-way bf16 split: x = h + m + l with each part bf16-exact. Products of
# bf16-exact values are exact in the PE's f32r mode, so a 6-combo expansion
# (dropping <1e-7 cross terms) gives fp32-class precision at full PE rate.
COMBOS = [(0, 0), (0, 1), (1, 0), (0, 2), (1, 1), (2, 0)]
NK = 4 * len(COMBOS)


def _split3(a):
    a = np.asarray(a, np.float64)
    h = a.astype(ml_dtypes.bfloat16).astype(np.float64)
    r = a - h
    m = r.astype(ml_dtypes.bfloat16).astype(np.float64)
    l = (r - m).astype(ml_dtypes.bfloat16).astype(np.float64)
    return [h, m, l]


def _face_arrays(prep, textures, faces):
    """Pack per-chunk coefficient/texture/scalar arrays for one chunk of 128
    face slots (index -1 = inert dummy)."""
    f = np.asarray(faces)
    dummy = f < 0
    fi = np.where(dummy, 0, f)

    def D(a):  # zero out dummies
        a = np.asarray(a, np.float64).copy()
        a[dummy] = 0.0
        return a

    # quantity order: U01,LD01,U12,LD12,U20,LD20,W0,W1,W2 -> coef[4, 9, 128]
    coef = np.zeros((4, 9, 128))
    for e in range(3):
        coef[:, 2 * e, :] = D(prep['UT'][e][fi]).T
        coef[:, 2 * e + 1, :] = D(prep['S2'][e][fi]).T
    coef[:, 6, :] = D(prep['W0c'][fi]).T
    coef[:, 7, :] = D(prep['W1c'][fi]).T
    # dummies: W0=W1=-1 (outside, wc2=1), LD=10 (dist 10 -> prob 0),
    # iz=0.011 -> zp~90.9 -> zn~0.092 (never the argmax), halfL=0.5
    coef[0, 1, dummy] = 10.0
    coef[0, 3, dummy] = 10.0
    coef[0, 5, dummy] = 10.0
    coef[0, 6, dummy] = -1.0
    coef[0, 7, dummy] = -1.0
    coef[:, 8, :] = -coef[:, 6, :] - coef[:, 7, :]
    coef[0, 8, :] += 1.0                                   # w2 = 1 - w0 - w1
    cs = _split3(coef)
    coefk = np.zeros((NK, 9, 128), np.float32)
    for t, (ci, bi) in enumerate(COMBOS):
        coefk[4 * t:4 * t + 4] = cs[ci].astype(np.float32)

    tex = np.asarray(textures, np.float64)[0][fi]          # [128,3,3] (k,c)
    tex[dummy] = 0.0

    scal = np.zeros((128, 9))
    izf = prep['iz'][fi]
    izf[dummy] = 0.011
    scal[:, 0:3] = izf
    hlf = prep['HL'][:, fi].T
    hlf[dummy] = 0.5
    scal[:, 3:6] = hlf
    scal[:, 6:9] = -hlf
    return coefk, tex, scal


def _build_program(pattern):
    """One SPMD Bass program; chunk counts per position given by pattern."""
    totc = sum(pattern)
    kmax = max(pattern)
    nc = bacc.Bacc("TRN2", target_bir_lowering=False, debug=False,
                   num_devices=NCORES)
    d_coef = nc.dram_tensor("coef", [totc, 24, 9 * 128], F32R, kind="ExternalInput")
    d_basis = nc.dram_tensor("basis", [NPOS, 24, TP], F32R, kind="ExternalInput")
    d_tex = nc.dram_tensor("tex", [128, totc * 9], FP, kind="ExternalInput")
    d_scal = nc.dram_tensor("scal", [128, totc * 9], FP, kind="ExternalInput")
    d_out = nc.dram_tensor("out6", [6, NPOS * TP], FP, kind="ExternalOutput")

    with ExitStack() as ctx:
        tc = ctx.enter_context(tile.TileContext(nc))
        const = ctx.enter_context(tc.tile_pool(name="const", bufs=1))
        stage = ctx.enter_context(tc.tile_pool(name="stage", bufs=2))
        basp = ctx.enter_context(tc.tile_pool(name="basp", bufs=2))
        work = ctx.enter_context(tc.tile_pool(name="work", bufs=1))
        store = ctx.enter_context(tc.tile_pool(name="store", bufs=3))
        zm = ctx.enter_context(tc.tile_pool(name="zm", bufs=2))
        qp = ctx.enter_context(tc.tile_pool(name="qp", bufs=6, space="PSUM"))
        accp = ctx.enter_context(tc.tile_pool(name="accp", bufs=2, space="PSUM"))

        ident = const.tile([128, 128], FP)
        make_identity(nc, ident)
        onesc = const.tile([128, 1], BF)
        nc.vector.memset(onesc, 1.0)
        onesf = const.tile([128, 1], FP)
        nc.vector.memset(onesf, 1.0)
        b_sqrt = const.tile([128, 1], FP)
        nc.vector.memset(b_sqrt, 1e-12)
        b_ln = const.tile([128, 1], FP)
        nc.vector.memset(b_ln, 1e-30)
        tex_sb = const.tile([128, totc * 9], FP)
        nc.sync.dma_start(out=tex_sb, in_=d_tex[:, :])
        scal_sb = const.tile([128, totc * 9], FP)
        nc.sync.dma_start(out=scal_sb, in_=d_scal[:, :])

        jj = 0
        for pos in range(NPOS):
            K = pattern[pos]
            bas = basp.tile([24, TP], F32R, tag="bas")
            nc.sync.dma_start(out=bas, in_=d_basis[pos, :, :])

            zn_st = store.tile([128, kmax, TP], FP, tag="zn_st")
            pv_st = store.tile([128, kmax, TP], FP, tag="pv_st")
            wn_st = [store.tile([128, kmax, TP], FP, tag=f"wn{k}_st",
                                 name=f"wn{k}_st") for k in range(3)]
            d2_st = store.tile([128, kmax, TP], FP, tag="d2_st")
            sb_st = store.tile([128, kmax, TP], mybir.dt.uint32, tag="sb_st")
            acc = accp.tile([65, TP], FP, tag="acc")

            # ---- phase 1a: per chunk, everything except sqrt/sigmoid/ln ----
            for j in range(K):
                cj = jj + j
                st = stage.tile([24, 9 * 128], F32R, tag="st")
                nc.sync.dma_start(out=st, in_=d_coef[cj, :, :])
                q = [qp.tile([128, TP], FP, tag="q", name=f"q{qi}")
                     for qi in range(9)]
                for qi in range(9):
                    nc.tensor.matmul(q[qi], st[:, qi * 128:(qi + 1) * 128],
                                     bas, start=True, stop=True)
                sc = lambda i: scal_sb[:, cj * 9 + i: cj * 9 + i + 1]

                # d2_e = LD^2 + max(|U - L/2| - L/2, 0)^2
                ru = [work.tile([128, TP], FP, tag=f"ru{e}", name=f"ru{e}")
                      for e in range(3)]
                tt = [work.tile([128, TP], FP, tag=f"t{e}", name=f"t{e}")
                      for e in range(3)]
                for e in range(3):
                    nc.scalar.activation(ru[e], q[2 * e + 1], AF.Square)
                    nc.scalar.activation(tt[e], q[2 * e], AF.Identity,
                                         bias=sc(6 + e))        # U - L/2
                    nc.vector.tensor_scalar(
                        out=tt[e].bitcast(mybir.dt.uint32),
                        in0=tt[e].bitcast(mybir.dt.uint32),
                        scalar1=0x7fffffff, scalar2=None,
                        op0=AL.bitwise_and)                     # abs
                    nc.scalar.activation(tt[e], tt[e], AF.Relu,
                                         bias=sc(6 + e))        # overshoot
                    nc.scalar.activation(tt[e], tt[e], AF.Square)
                    nc.vector.tensor_tensor(out=ru[e], in0=ru[e], in1=tt[e],
                                            op=AL.add)
                nc.vector.tensor_tensor(out=ru[0], in0=ru[0], in1=ru[1],
                                        op=AL.min)
                nc.vector.tensor_tensor(out=d2_st[:, j, :], in0=ru[0],
                                        in1=ru[2], op=AL.min)

                cw0 = work.tile([128, TP], FP, tag="cw0")
                nc.scalar.activation(cw0, q[6], AF.Copy)
                cw1 = work.tile([128, TP], FP, tag="cw1")
                nc.scalar.activation(cw1, q[7], AF.Copy)
                cw2 = work.tile([128, TP], FP, tag="cw2")
                nc.scalar.activation(cw2, q[8], AF.Copy)
                m1 = work.tile([128, TP], FP, tag="m1")
                nc.vector.tensor_tensor(out=m1, in0=cw0, in1=cw1, op=AL.min)
                nc.vector.tensor_tensor(out=m1, in0=m1, in1=cw2, op=AL.min)
                nc.vector.tensor_scalar(out=sb_st[:, j, :],
                                        in0=m1.bitcast(mybir.dt.uint32),
                                        scalar1=0x80000000, scalar2=None,
                                        op0=AL.bitwise_and)

                wc0, wc1, wc2 = cw0, cw1, cw2
                nc.vector.tensor_scalar(out=wc0, in0=cw0, scalar1=0.0,
                                        scalar2=1.0, op0=AL.max, op1=AL.min)
                nc.vector.tensor_scalar(out=wc1, in0=cw1, scalar1=0.0,
                                        scalar2=1.0, op0=AL.max, op1=AL.min)
                nc.vector.tensor_scalar(out=wc2, in0=cw2, scalar1=0.0,
                                        scalar2=1.0, op0=AL.max, op1=AL.min)
                s01 = work.tile([128, TP], FP, tag="s01")
                nc.vector.tensor_tensor(out=s01, in0=wc0, in1=wc1, op=AL.add)
                nc.vector.tensor_tensor(out=s01, in0=s01, in1=wc2, op=AL.add)
                invs = work.tile([128, TP], FP, tag="invs")
                nc.vector.reciprocal_approx_fast(out=invs, in_=s01)
                r1 = work.tile([128, TP], FP, tag="r1")
                nc.vector.tensor_scalar(out=r1, in0=wc0, scalar1=sc(0),
                                        scalar2=None, op0=AL.mult)
                nc.vector.scalar_tensor_tensor(out=r1, in0=wc1, scalar=sc(1),
                                               in1=r1, op0=AL.mult, op1=AL.add)
                nc.vector.scalar_tensor_tensor(out=r1, in0=wc2, scalar=sc(2),
                                               in1=r1, op0=AL.mult, op1=AL.add)
                nc.vector.tensor_tensor(out=r1, in0=r1, in1=invs, op=AL.mult)
                nc.vector.reciprocal_approx_fast(out=s01, in_=r1)   # zp
                nc.vector.tensor_scalar(out=zn_st[:, j, :], in0=s01,
                                        scalar1=-1.0 / (FAR - NEAR),
                                        scalar2=FAR / (FAR - NEAR),
                                        op0=AL.mult, op1=AL.add)
                for k, wck in enumerate([wc0, wc1, wc2]):
                    nc.gpsimd.tensor_tensor(out=wn_st[k][:, j, :], in0=wck,
                                            in1=invs, op=AL.mult)

            # ---- phase 1b: batched sqrt / sign / sigmoid / ln ----
            for j in range(K):
                nc.scalar.activation(d2_st[:, j, :], d2_st[:, j, :], AF.Sqrt,
                                     bias=b_sqrt)
            for j in range(K):
                nc.vector.tensor_tensor(
                    out=d2_st[:, j, :].bitcast(mybir.dt.uint32),
                    in0=d2_st[:, j, :].bitcast(mybir.dt.uint32),
                    in1=sb_st[:, j, :], op=AL.bitwise_or)
            for j in range(K):
                nc.scalar.activation(pv_st[:, j, :], d2_st[:, j, :],
                                     AF.Sigmoid, scale=1.0 / SIGMA)
            for j in range(K):
                q1m = work.tile([128, TP], FP, tag="q1m")
                nc.vector.tensor_scalar(out=q1m, in0=pv_st[:, j, :],
                                        scalar1=-1.0, scalar2=1.0,
                                        op0=AL.mult, op1=AL.add)
                lq = work.tile([128, TP], FP, tag="lq")
                nc.scalar.activation(lq, q1m, AF.Ln, bias=b_ln)
                nc.tensor.matmul(acc[64:65, :], onesf[:, 0:1], lq,
                                 start=(j == 0), stop=(j == K - 1))

            # ---- phase 2: per-pixel zmax (negated) via transposes ----
            zmc = zm.tile([128, 4], FP, tag="zmc")
            dts = [zm.tile([128, kmax * 128], FP, tag=f"dt{s}", name=f"dt{s}")
                   for s in range(4)]
            for s in range(4):
                znt = qp.tile([128, kmax * 128], FP, tag="q")
                for j in range(K):
                    nc.tensor.transpose(
                        znt[:, j * 128:(j + 1) * 128],
                        zn_st[:, j, s * 128:(s + 1) * 128], ident)
                nc.vector.tensor_reduce(
                    out=zmc[:, s:s + 1], in_=znt[:, 0:K * 128],
                    axis=mybir.AxisListType.X, op=AL.max)
                nc.vector.tensor_scalar(out=zmc[:, s:s + 1],
                                        in0=zmc[:, s:s + 1], scalar1=EPS,
                                        scalar2=-1.0, op0=AL.max, op1=AL.mult)
                nc.scalar.activation(dts[s][:, 0:K * 128], znt[:, 0:K * 128],
                                     AF.Identity, bias=zmc[:, s:s + 1])

            # ---- phase 3: exp weights, rgb/dsum accumulation ----
            for j in range(K):
                cj = jj + j
                dp = qp.tile([128, TP], FP, tag="q")
                for s in range(4):
                    nc.tensor.transpose(dp[:, s * 128:(s + 1) * 128],
                                        dts[s][:, j * 128:(j + 1) * 128],
                                        ident)
                d = work.tile([128, TP], FP, tag="d")
                nc.scalar.activation(d, dp, AF.Exp, scale=1.0 / GAMMA)
                nc.vector.tensor_tensor(out=d, in0=pv_st[:, j, :], in1=d,
                                        op=AL.mult)             # wexp
                for k in range(3):
                    g = work.tile([128, TP], FP, tag="g", bufs=3)
                    eng = nc.vector if k == 0 else nc.gpsimd
                    eng.tensor_tensor(out=g, in0=d,
                                      in1=wn_st[k][:, j, :], op=AL.mult)
                    nc.tensor.matmul(
                        acc[0:3, :],
                        tex_sb[:, cj * 9 + k * 3: cj * 9 + (k + 1) * 3], g,
                        start=(j == 0 and k == 0),
                        stop=(j == K - 1 and k == 2))
                nc.tensor.matmul(acc[32:33, :], onesf[:, 0:1], d,
                                 start=(j == 0), stop=(j == K - 1))

            o6 = zm.tile([65, TP], FP, tag="o6")
            nc.scalar.activation(o6[0:3, :], acc[0:3, :], AF.Copy)
            nc.scalar.activation(o6[32:33, :], acc[32:33, :], AF.Copy)
            nc.scalar.activation(o6[64:65, :], acc[64:65, :], AF.Copy)
            nc.sync.dma_start(out=d_out[0:3, pos * TP:(pos + 1) * TP],
                              in_=o6[0:3, :])
            nc.sync.dma_start(out=d_out[3:4, pos * TP:(pos + 1) * TP],
                              in_=o6[32:33, :])
            nc.sync.dma_start(out=d_out[4:5, pos * TP:(pos + 1) * TP],
                              in_=o6[64:65, :])
            nc.sync.dma_start(
                out=d_out[5, pos * TP:(pos + 1) * TP].rearrange(
                    "(s p) -> p s", s=4),
                in_=zmc)
            jj += K
    nc.compile()
    return nc


def kernel(face_vertices, face_textures):
    prep = _host_prep(face_vertices)
    pattern, assign = _cull_and_balance(prep)
    totc = sum(pattern)

    pix = ((np.arange(H, dtype=np.float64) + 0.5) / H) * 2.0 - 1.0
    in_maps = []
    for c in range(NCORES):
        coef = np.zeros((totc, NK, 9 * 128), np.float32)
        tex = np.zeros((128, totc * 9), np.float32)
        scal = np.zeros((128, totc * 9), np.float32)
        basis = np.zeros((NPOS, NK, TP), np.float32)
        jj = 0
        for pos in range(NPOS):
            b, faces = assign[c][pos]
            yb, xb = b // 2, b % 2
            py = np.repeat(pix[4 * yb:4 * yb + 4], 128)
            px = np.tile(pix[128 * xb:128 * xb + 128], 4)
            b4 = np.stack([np.ones(TP), px, py, px ** 2 + py ** 2])
            bs = _split3(b4)
            for t, (ci, bi) in enumerate(COMBOS):
                basis[pos, 4 * t:4 * t + 4] = bs[bi].astype(np.float32)
            for j in range(pattern[pos]):
                cf, tx, sl = _face_arrays(prep, face_textures,
                                          faces[j * 128:(j + 1) * 128])
                coef[jj] = cf.reshape(NK, 9 * 128)
                tex[:, jj * 9:(jj + 1) * 9] = tx.reshape(128, 9)
                scal[:, jj * 9:(jj + 1) * 9] = sl
                jj += 1
        in_maps.append({"coef": coef, "basis": basis, "tex": tex, "scal": scal})

    nc = _build_program(pattern)
    global LAST_RESULT
    if TRACE:
        res = run_bass_kernel_spmd(nc, in_maps, core_ids=list(range(NCORES)),
                                   trace=True)
    else:
        res = run_bass_kernel_spmd(nc, in_maps, core_ids=list(range(NCORES)))
    LAST_RESULT = res

    out = np.zeros((1, 4, H, W), np.float32)
    for c in range(NCORES):
        o6 = res.results[c]["out6"]                        # [6, NPOS*TP]
        for pos in range(NPOS):
            b, _ = assign[c][pos]
            yb, xb = b // 2, b % 2
            seg = o6[:, pos * TP:(pos + 1) * TP]
            zmax = -seg[5]
            wbg = np.exp((EPS - zmax) / GAMMA).astype(np.float32)
            dsum = seg[3] + wbg
            rgb = seg[0:3] / dsum[None]
            alpha = 1.0 - np.exp(seg[4])
            ys = slice(4 * yb, 4 * yb + 4)
            xs = slice(128 * xb, 128 * xb + 128)
            out[0, 0:3, ys, xs] = rgb.reshape(3, 4, 128)
            out[0, 3, ys, xs] = alpha.reshape(4, 128)
    return out


# revision 22
# speedup vs baseline: 1.3827x; 1.0022x over previous
"""SoftRas-style soft rasterizer on 8 Trainium2 NeuronCores.

Strategy:
- All per-(face,pixel) affine quantities (barycentric w0/w1, edge projections
  u/l2, squared vertex distances) are produced by TensorE matmuls against the
  pixel basis [1, px, py, px^2+py^2] (K=4).
- The nonlinear chain (clip/sqrt/sigmoid/reciprocal/exp/log) runs on
  VectorE/ScalarE with faces on partitions (128/chunk), pixels on the free dim.
- Per-pixel max over faces (softmax shift) via PE transposes + free-dim max.
- Face-direction reductions (rgb accumulation, dsum, sum log(1-p)) via PE
  matmuls against textures / ones.
- Host (numpy): per-face coefficient prep, per-tile face culling, per-core
  load balancing (every core gets an identical chunk-count pattern so one
  SPMD program serves all 8 cores), final divide + alpha exponentiation.
"""
import sys
sys.path.insert(0, '/opt/trn_rl_repo')
import numpy as np
import ml_dtypes
from contextlib import ExitStack

import concourse.bass as bass
import concourse.bacc as bacc
import concourse.tile as tile
import concourse.mybir as mybir
from concourse.bass_utils import run_bass_kernel_spmd
from concourse.masks import make_identity

TRACE = False
LAST_RESULT = None

F_TOT = 512
H = W = 256
NCORES = 8
TP = 512              # pixels per tile position (2 image rows)
NPOS = (H * W) // (NCORES * TP)   # 16 tile positions per core
SIGMA = 1e-2
GAMMA = 1e-3
EPS = 1e-3
NEAR, FAR = 1.0, 100.0
FP = mybir.dt.float32
F32R = mybir.dt.float32r
BF = mybir.dt.bfloat16
AL = mybir.AluOpType
AF = mybir.ActivationFunctionType


def _host_prep(face_vertices):
    """Per-face coefficients in basis [1, px, py, r2], fp64 -> fp32."""
    fv = np.asarray(face_vertices, np.float64)[0]          # [F,3,3]
    F = fv.shape[0]
    x = fv[:, :, 0]; y = fv[:, :, 1]; z = fv[:, :, 2]
    x0, x1, x2 = x[:, 0], x[:, 1], x[:, 2]
    y0, y1, y2 = y[:, 0], y[:, 1], y[:, 2]

    den = (y1 - y2) * (x0 - x2) + (x2 - x1) * (y0 - y2)
    den = np.where(np.abs(den) < 1e-10, 1e-10, den)
    W0c = np.stack([(-(y1 - y2) * x2 - (x2 - x1) * y2) / den,
                    (y1 - y2) / den, (x2 - x1) / den, np.zeros(F)], -1)
    W1c = np.stack([(-(y2 - y0) * x2 - (x0 - x2) * y2) / den,
                    (y2 - y0) / den, (x0 - x2) / den, np.zeros(F)], -1)

    anchors = [(x0, y0), (x1, y1), (x2, y2)]
    pairs = [(0, 1), (1, 2), (2, 0)]
    # per edge: U = ((p-a).d)/|d| (along-line coord), LD = cross(p-a, d)/|d|
    # (signed line distance). d2seg = LD^2 + max(|U - L/2| - L/2, 0)^2.
    UT = np.zeros((3, F, 4)); S2 = np.zeros((3, F, 4)); HL = np.zeros((3, F))
    for e, (ia, ib) in enumerate(pairs):
        ax, ay = anchors[ia]; bx, by = anchors[ib]
        dx, dy = bx - ax, by - ay
        L = np.sqrt(np.maximum(dx * dx + dy * dy, 1e-12))
        iL = 1.0 / L
        UT[e, :, 0] = (-ax * dx - ay * dy) * iL
        UT[e, :, 1] = dx * iL
        UT[e, :, 2] = dy * iL
        S2[e, :, 0] = (ay * dx - ax * dy) * iL
        S2[e, :, 1] = dy * iL
        S2[e, :, 2] = -dx * iL
        HL[e] = L / 2.0
    iz = 1.0 / z
    zmin = z.min(1); zmax = z.max(1)
    assert z.min() > NEAR + 0.05 and z.max() < FAR - 0.05, \
        "kernel fast path assumes all vertex depths strictly inside (NEAR,FAR)"
    return dict(W0c=W0c, W1c=W1c, UT=UT, S2=S2, HL=HL, iz=iz,
                ymin=y.min(1), ymax=y.max(1), xmin=x.min(1), xmax=x.max(1),
                zmin=zmin, zmax=zmax)


def _cull_and_balance(prep):
    """Per tile (4 rows x 128 px), the kept-face list; balanced so all cores
    share one chunk-count pattern. Returns (pattern, assign) where
    assign[core][pos] = (tile_index, face_index_array padded with -1)."""
    nyb = H // 4
    pixc = ((np.arange(H) + 0.5) / H) * 2.0 - 1.0
    tiles = []
    for yb in range(nyb):
        for xb in range(2):
            tiles.append((pixc[4 * yb], pixc[4 * yb + 3],
                          pixc[128 * xb], pixc[128 * xb + 127]))
    tiles = np.array(tiles)                                # [nb, 4]
    nb = len(tiles)
    ygap = np.maximum(0.0, np.maximum(
        prep['ymin'][None, :] - tiles[:, 1:2],
        tiles[:, 0:1] - prep['ymax'][None, :]))
    xgap = np.maximum(0.0, np.maximum(
        prep['xmin'][None, :] - tiles[:, 3:4],
        tiles[:, 2:3] - prep['xmax'][None, :]))
    gap = np.sqrt(xgap ** 2 + ygap ** 2)
    znUB = (FAR - prep['zmin']) / (FAR - NEAR)
    znLB = (FAR - prep['zmax']) / (FAR - NEAR)
    D = znLB.max()
    keep = (gap < 0.17) | ((gap / SIGMA) + (D - znUB) / GAMMA < 87.0)  # [nb,F]

    counts = np.maximum(1, np.ceil(keep.sum(1) / 128).astype(int))
    order = np.argsort(-counts, kind='stable')             # bands, desc count
    pattern = [int(counts[order[p * NCORES]]) for p in range(NPOS)]
    assign = [[None] * NPOS for _ in range(NCORES)]
    for p in range(NPOS):
        for c in range(NCORES):
            b = int(order[p * NCORES + c])
            faces = np.nonzero(keep[b])[0]
            pad = pattern[p] * 128 - len(faces)
            assert pad >= 0
            faces = np.concatenate([faces, -np.ones(pad, np.int64)])
            assign[c][p] = (b, faces)
    return pattern, assign


# 3-way bf16 split: x = h + m + l with each part bf16-exact. Products of
# bf16-exact values are exact in the PE's f32r mode, so a 6-combo expansion
# (dropping <1e-7 cross terms) gives fp32-class precision at full PE rate.
COMBOS = [(0, 0), (0, 1), (1, 0), (0, 2), (1, 1), (2, 0)]
NK = 4 * len(COMBOS)


def _split3(a):
    a = np.asarray(a, np.float64)
    h = a.astype(ml_dtypes.bfloat16).astype(np.float64)
    r = a - h
    m = r.astype(ml_dtypes.bfloat16).astype(np.float64)
    l = (r - m).astype(ml_dtypes.bfloat16).astype(np.float64)
    return [h, m, l]


def _face_arrays(prep, textures, faces):
    """Pack per-chunk coefficient/texture/scalar arrays for one chunk of 128
    face slots (index -1 = inert dummy)."""
    f = np.asarray(faces)
    dummy = f < 0
    fi = np.where(dummy, 0, f)

    def D(a):  # zero out dummies
        a = np.asarray(a, np.float64).copy()
        a[dummy] = 0.0
        return a

    # quantity order: U01,LD01,U12,LD12,U20,LD20,W0,W1,W2 -> coef[4, 9, 128]
    coef = np.zeros((4, 9, 128))
    for e in range(3):
        coef[:, 2 * e, :] = D(prep['UT'][e][fi]).T
        coef[:, 2 * e + 1, :] = D(prep['S2'][e][fi]).T
    coef[:, 6, :] = D(prep['W0c'][fi]).T
    coef[:, 7, :] = D(prep['W1c'][fi]).T
    # dummies: W0=W1=-1 (outside, wc2=1), LD=10 (dist 10 -> prob 0),
    # iz=0.011 -> zp~90.9 -> zn~0.092 (never the argmax), halfL=0.5
    coef[0, 1, dummy] = 10.0
    coef[0, 3, dummy] = 10.0
    coef[0, 5, dummy] = 10.0
    coef[0, 6, dummy] = -1.0
    coef[0, 7, dummy] = -1.0
    coef[:, 8, :] = -coef[:, 6, :] - coef[:, 7, :]
    coef[0, 8, :] += 1.0                                   # w2 = 1 - w0 - w1
    cs = _split3(coef)
    coefk = np.zeros((NK, 9, 128), np.float32)
    for t, (ci, bi) in enumerate(COMBOS):
        coefk[4 * t:4 * t + 4] = cs[ci].astype(np.float32)

    tex = np.asarray(textures, np.float64)[0][fi]          # [128,3,3] (k,c)
    tex[dummy] = 0.0

    scal = np.zeros((128, 9))
    izf = prep['iz'][fi]
    izf[dummy] = 0.011
    scal[:, 0:3] = izf
    hlf = prep['HL'][:, fi].T
    hlf[dummy] = 0.5
    scal[:, 3:6] = hlf
    scal[:, 6:9] = -hlf
    return coefk, tex, scal


def _build_program(pattern):
    """One SPMD Bass program; chunk counts per position given by pattern."""
    totc = sum(pattern)
    kmax = max(pattern)
    nc = bacc.Bacc("TRN2", target_bir_lowering=False, debug=False,
                   num_devices=NCORES)
    d_coef = nc.dram_tensor("coef", [totc, 24, 9 * 128], F32R, kind="ExternalInput")
    d_basis = nc.dram_tensor("basis", [NPOS, 24, TP], F32R, kind="ExternalInput")
    d_tex = nc.dram_tensor("tex", [128, totc * 9], FP, kind="ExternalInput")
    d_scal = nc.dram_tensor("scal", [128, totc * 9], FP, kind="ExternalInput")
    d_out = nc.dram_tensor("out6", [6, NPOS * TP], FP, kind="ExternalOutput")

    with ExitStack() as ctx:
        tc = ctx.enter_context(tile.TileContext(nc))
        const = ctx.enter_context(tc.tile_pool(name="const", bufs=1))
        stage = ctx.enter_context(tc.tile_pool(name="stage", bufs=3))
        basp = ctx.enter_context(tc.tile_pool(name="basp", bufs=3))
        work = ctx.enter_context(tc.tile_pool(name="work", bufs=2))
        store = ctx.enter_context(tc.tile_pool(name="store", bufs=3))
        zm = ctx.enter_context(tc.tile_pool(name="zm", bufs=3))
        qp = ctx.enter_context(tc.tile_pool(name="qp", bufs=6, space="PSUM"))
        accp = ctx.enter_context(tc.tile_pool(name="accp", bufs=2, space="PSUM"))

        ident = const.tile([128, 128], FP)
        make_identity(nc, ident)
        onesc = const.tile([128, 1], BF)
        nc.vector.memset(onesc, 1.0)
        onesf = const.tile([128, 1], FP)
        nc.vector.memset(onesf, 1.0)
        b_sqrt = const.tile([128, 1], FP)
        nc.vector.memset(b_sqrt, 1e-12)
        b_ln = const.tile([128, 1], FP)
        nc.vector.memset(b_ln, 1e-30)
        tex_sb = const.tile([128, totc * 9], FP)
        nc.sync.dma_start(out=tex_sb, in_=d_tex[:, :])
        scal_sb = const.tile([128, totc * 9], FP)
        nc.sync.dma_start(out=scal_sb, in_=d_scal[:, :])

        jj = 0
        for pos in range(NPOS):
            K = pattern[pos]
            bas = basp.tile([24, TP], F32R, tag="bas")
            nc.sync.dma_start(out=bas, in_=d_basis[pos, :, :])

            zn_st = store.tile([128, kmax, TP], FP, tag="zn_st")
            pv_st = store.tile([128, kmax, TP], FP, tag="pv_st")
            wn_st = [store.tile([128, kmax, TP], FP, tag=f"wn{k}_st",
                                 name=f"wn{k}_st") for k in range(3)]
            d2_st = store.tile([128, kmax, TP], FP, tag="d2_st")
            sb_st = store.tile([128, kmax, TP], mybir.dt.uint32, tag="sb_st")
            acc = accp.tile([65, TP], FP, tag="acc")

            # ---- phase 1a: per chunk, everything except sqrt/sigmoid/ln ----
            for j in range(K):
                cj = jj + j
                st = stage.tile([24, 9 * 128], F32R, tag="st")
                nc.sync.dma_start(out=st, in_=d_coef[cj, :, :])
                q = [qp.tile([128, TP], FP, tag="q", name=f"q{qi}")
                     for qi in range(9)]
                for qi in range(9):
                    nc.tensor.matmul(q[qi], st[:, qi * 128:(qi + 1) * 128],
                                     bas, start=True, stop=True)
                sc = lambda i: scal_sb[:, cj * 9 + i: cj * 9 + i + 1]

                # d2_e = LD^2 + max(|U - L/2| - L/2, 0)^2
                ru = [work.tile([128, TP], FP, tag=f"ru{e}", name=f"ru{e}")
                      for e in range(3)]
                tt = [work.tile([128, TP], FP, tag=f"t{e}", name=f"t{e}")
                      for e in range(3)]
                for e in range(3):
                    nc.scalar.activation(ru[e], q[2 * e + 1], AF.Square)
                    nc.scalar.activation(tt[e], q[2 * e], AF.Identity,
                                         bias=sc(6 + e))        # U - L/2
                    nc.vector.tensor_scalar(
                        out=tt[e].bitcast(mybir.dt.uint32),
                        in0=tt[e].bitcast(mybir.dt.uint32),
                        scalar1=0x7fffffff, scalar2=None,
                        op0=AL.bitwise_and)                     # abs
                    nc.scalar.activation(tt[e], tt[e], AF.Relu,
                                         bias=sc(6 + e))        # overshoot
                    nc.scalar.activation(tt[e], tt[e], AF.Square)
                    nc.vector.tensor_tensor(out=ru[e], in0=ru[e], in1=tt[e],
                                            op=AL.add)
                nc.vector.tensor_tensor(out=ru[0], in0=ru[0], in1=ru[1],
                                        op=AL.min)
                nc.vector.tensor_tensor(out=d2_st[:, j, :], in0=ru[0],
                                        in1=ru[2], op=AL.min)

                cw0 = work.tile([128, TP], FP, tag="cw0")
                nc.scalar.activation(cw0, q[6], AF.Copy)
                cw1 = work.tile([128, TP], FP, tag="cw1")
                nc.scalar.activation(cw1, q[7], AF.Copy)
                cw2 = work.tile([128, TP], FP, tag="cw2")
                nc.scalar.activation(cw2, q[8], AF.Copy)
                m1 = work.tile([128, TP], FP, tag="m1")
                nc.vector.tensor_tensor(out=m1, in0=cw0, in1=cw1, op=AL.min)
                nc.vector.tensor_tensor(out=m1, in0=m1, in1=cw2, op=AL.min)
                nc.vector.tensor_scalar(out=sb_st[:, j, :],
                                        in0=m1.bitcast(mybir.dt.uint32),
                                        scalar1=0x80000000, scalar2=None,
                                        op0=AL.bitwise_and)

                wc0, wc1, wc2 = cw0, cw1, cw2
                nc.vector.tensor_scalar(out=wc0, in0=cw0, scalar1=0.0,
                                        scalar2=1.0, op0=AL.max, op1=AL.min)
                nc.vector.tensor_scalar(out=wc1, in0=cw1, scalar1=0.0,
                                        scalar2=1.0, op0=AL.max, op1=AL.min)
                nc.vector.tensor_scalar(out=wc2, in0=cw2, scalar1=0.0,
                                        scalar2=1.0, op0=AL.max, op1=AL.min)
                s01 = work.tile([128, TP], FP, tag="s01")
                nc.vector.tensor_tensor(out=s01, in0=wc0, in1=wc1, op=AL.add)
                nc.vector.tensor_tensor(out=s01, in0=s01, in1=wc2, op=AL.add)
                invs = work.tile([128, TP], FP, tag="invs")
                nc.vector.reciprocal_approx_fast(out=invs, in_=s01)
                r1 = work.tile([128, TP], FP, tag="r1")
                nc.vector.tensor_scalar(out=r1, in0=wc0, scalar1=sc(0),
                                        scalar2=None, op0=AL.mult)
                nc.vector.scalar_tensor_tensor(out=r1, in0=wc1, scalar=sc(1),
                                               in1=r1, op0=AL.mult, op1=AL.add)
                nc.vector.scalar_tensor_tensor(out=r1, in0=wc2, scalar=sc(2),
                                               in1=r1, op0=AL.mult, op1=AL.add)
                nc.vector.tensor_tensor(out=r1, in0=r1, in1=invs, op=AL.mult)
                nc.vector.reciprocal_approx_fast(out=s01, in_=r1)   # zp
                nc.vector.tensor_scalar(out=zn_st[:, j, :], in0=s01,
                                        scalar1=-1.0 / (FAR - NEAR),
                                        scalar2=FAR / (FAR - NEAR),
                                        op0=AL.mult, op1=AL.add)
                for k, wck in enumerate([wc0, wc1, wc2]):
                    nc.gpsimd.tensor_tensor(out=wn_st[k][:, j, :], in0=wck,
                                            in1=invs, op=AL.mult)

            # ---- phase 1b: batched sqrt / sign / sigmoid / ln ----
            for j in range(K):
                nc.scalar.activation(d2_st[:, j, :], d2_st[:, j, :], AF.Sqrt,
                                     bias=b_sqrt)
            for j in range(K):
                nc.vector.tensor_tensor(
                    out=d2_st[:, j, :].bitcast(mybir.dt.uint32),
                    in0=d2_st[:, j, :].bitcast(mybir.dt.uint32),
                    in1=sb_st[:, j, :], op=AL.bitwise_or)
            for j in range(K):
                nc.scalar.activation(pv_st[:, j, :], d2_st[:, j, :],
                                     AF.Sigmoid, scale=1.0 / SIGMA)
            for j in range(K):
                q1m = work.tile([128, TP], FP, tag="q1m")
                nc.vector.tensor_scalar(out=q1m, in0=pv_st[:, j, :],
                                        scalar1=-1.0, scalar2=1.0,
                                        op0=AL.mult, op1=AL.add)
                lq = work.tile([128, TP], FP, tag="lq")
                nc.scalar.activation(lq, q1m, AF.Ln, bias=b_ln)
                nc.tensor.matmul(acc[64:65, :], onesf[:, 0:1], lq,
                                 start=(j == 0), stop=(j == K - 1))

            # ---- phase 2: per-pixel zmax (negated) via transposes ----
            zmc = zm.tile([128, 4], FP, tag="zmc")
            dts = [zm.tile([128, kmax * 128], FP, tag=f"dt{s}", name=f"dt{s}")
                   for s in range(4)]
            for s in range(4):
                znt = qp.tile([128, kmax * 128], FP, tag="q")
                for j in range(K):
                    nc.tensor.transpose(
                        znt[:, j * 128:(j + 1) * 128],
                        zn_st[:, j, s * 128:(s + 1) * 128], ident)
                nc.vector.tensor_reduce(
                    out=zmc[:, s:s + 1], in_=znt[:, 0:K * 128],
                    axis=mybir.AxisListType.X, op=AL.max)
                nc.vector.tensor_scalar(out=zmc[:, s:s + 1],
                                        in0=zmc[:, s:s + 1], scalar1=EPS,
                                        scalar2=-1.0, op0=AL.max, op1=AL.mult)
                nc.scalar.activation(dts[s][:, 0:K * 128], znt[:, 0:K * 128],
                                     AF.Identity, bias=zmc[:, s:s + 1])

            # ---- phase 3: exp weights, rgb/dsum accumulation ----
            for j in range(K):
                cj = jj + j
                dp = qp.tile([128, TP], FP, tag="q")
                for s in range(4):
                    nc.tensor.transpose(dp[:, s * 128:(s + 1) * 128],
                                        dts[s][:, j * 128:(j + 1) * 128],
                                        ident)
                d = work.tile([128, TP], FP, tag="d")
                nc.scalar.activation(d, dp, AF.Exp, scale=1.0 / GAMMA)
                nc.vector.tensor_tensor(out=d, in0=pv_st[:, j, :], in1=d,
                                        op=AL.mult)             # wexp
                for k in range(3):
                    g = work.tile([128, TP], FP, tag="g", bufs=3)
                    eng = nc.vector if k == 0 else nc.gpsimd
                    eng.tensor_tensor(out=g, in0=d,
                                      in1=wn_st[k][:, j, :], op=AL.mult)
                    nc.tensor.matmul(
                        acc[0:3, :],
                        tex_sb[:, cj * 9 + k * 3: cj * 9 + (k + 1) * 3], g,
                        start=(j == 0 and k == 0),
                        stop=(j == K - 1 and k == 2))
                nc.tensor.matmul(acc[32:33, :], onesf[:, 0:1], d,
                                 start=(j == 0), stop=(j == K - 1))

            o6 = zm.tile([65, TP], FP, tag="o6")
            nc.scalar.activation(o6[0:3, :], acc[0:3, :], AF.Copy)
            nc.scalar.activation(o6[32:33, :], acc[32:33, :], AF.Copy)
            nc.scalar.activation(o6[64:65, :], acc[64:65, :], AF.Copy)
            nc.sync.dma_start(out=d_out[0:3, pos * TP:(pos + 1) * TP],
                              in_=o6[0:3, :])
            nc.sync.dma_start(out=d_out[3:4, pos * TP:(pos + 1) * TP],
                              in_=o6[32:33, :])
            nc.sync.dma_start(out=d_out[4:5, pos * TP:(pos + 1) * TP],
                              in_=o6[64:65, :])
            nc.sync.dma_start(
                out=d_out[5, pos * TP:(pos + 1) * TP].rearrange(
                    "(s p) -> p s", s=4),
                in_=zmc)
            jj += K
    nc.compile()
    return nc


def kernel(face_vertices, face_textures):
    prep = _host_prep(face_vertices)
    pattern, assign = _cull_and_balance(prep)
    totc = sum(pattern)

    pix = ((np.arange(H, dtype=np.float64) + 0.5) / H) * 2.0 - 1.0
    in_maps = []
    for c in range(NCORES):
        coef = np.zeros((totc, NK, 9 * 128), np.float32)
        tex = np.zeros((128, totc * 9), np.float32)
        scal = np.zeros((128, totc * 9), np.float32)
        basis = np.zeros((NPOS, NK, TP), np.float32)
        jj = 0
        for pos in range(NPOS):
            b, faces = assign[c][pos]
            yb, xb = b // 2, b % 2
            py = np.repeat(pix[4 * yb:4 * yb + 4], 128)
            px = np.tile(pix[128 * xb:128 * xb + 128], 4)
            b4 = np.stack([np.ones(TP), px, py, px ** 2 + py ** 2])
            bs = _split3(b4)
            for t, (ci, bi) in enumerate(COMBOS):
                basis[pos, 4 * t:4 * t + 4] = bs[bi].astype(np.float32)
            for j in range(pattern[pos]):
                cf, tx, sl = _face_arrays(prep, face_textures,
                                          faces[j * 128:(j + 1) * 128])
                coef[jj] = cf.reshape(NK, 9 * 128)
                tex[:, jj * 9:(jj + 1) * 9] = tx.reshape(128, 9)
                scal[:, jj * 9:(jj + 1) * 9] = sl
                jj += 1
        in_maps.append({"coef": coef, "basis": basis, "tex": tex, "scal": scal})

    nc = _build_program(pattern)
    global LAST_RESULT
    if TRACE:
        res = run_bass_kernel_spmd(nc, in_maps, core_ids=list(range(NCORES)),
                                   trace=True)
    else:
        res = run_bass_kernel_spmd(nc, in_maps, core_ids=list(range(NCORES)))
    LAST_RESULT = res

    out = np.zeros((1, 4, H, W), np.float32)
    for c in range(NCORES):
        o6 = res.results[c]["out6"]                        # [6, NPOS*TP]
        for pos in range(NPOS):
            b, _ = assign[c][pos]
            yb, xb = b // 2, b % 2
            seg = o6[:, pos * TP:(pos + 1) * TP]
            zmax = -seg[5]
            wbg = np.exp((EPS - zmax) / GAMMA).astype(np.float32)
            dsum = seg[3] + wbg
            rgb = seg[0:3] / dsum[None]
            alpha = 1.0 - np.exp(seg[4])
            ys = slice(4 * yb, 4 * yb + 4)
            xs = slice(128 * xb, 128 * xb + 128)
            out[0, 0:3, ys, xs] = rgb.reshape(3, 4, 128)
            out[0, 3, ys, xs] = alpha.reshape(4, 128)
    return out


# revision 23
# speedup vs baseline: 1.8206x; 1.3167x over previous
"""SoftRas-style soft rasterizer on 8 Trainium2 NeuronCores.

Strategy:
- All per-(face,pixel) affine quantities (barycentric w0/w1, edge projections
  u/l2, squared vertex distances) are produced by TensorE matmuls against the
  pixel basis [1, px, py, px^2+py^2] (K=4).
- The nonlinear chain (clip/sqrt/sigmoid/reciprocal/exp/log) runs on
  VectorE/ScalarE with faces on partitions (128/chunk), pixels on the free dim.
- Per-pixel max over faces (softmax shift) via PE transposes + free-dim max.
- Face-direction reductions (rgb accumulation, dsum, sum log(1-p)) via PE
  matmuls against textures / ones.
- Host (numpy): per-face coefficient prep, per-tile face culling, per-core
  load balancing (every core gets an identical chunk-count pattern so one
  SPMD program serves all 8 cores), final divide + alpha exponentiation.
"""
import sys
sys.path.insert(0, '/opt/trn_rl_repo')
import numpy as np
import ml_dtypes
from contextlib import ExitStack

import concourse.bass as bass
import concourse.bacc as bacc
import concourse.tile as tile
import concourse.mybir as mybir
from concourse.bass_utils import run_bass_kernel_spmd
from concourse.masks import make_identity

TRACE = False
LAST_RESULT = None

F_TOT = 512
H = W = 256
NCORES = 8
TP = 512              # pixels per tile position (2 image rows)
NPOS = (H * W) // (NCORES * TP)   # 16 tile positions per core
SIGMA = 1e-2
GAMMA = 1e-3
EPS = 1e-3
NEAR, FAR = 1.0, 100.0
FP = mybir.dt.float32
F32R = mybir.dt.float32r
BF = mybir.dt.bfloat16
AL = mybir.AluOpType
AF = mybir.ActivationFunctionType


def _host_prep(face_vertices):
    """Per-face coefficients in basis [1, px, py, r2], fp64 -> fp32."""
    fv = np.asarray(face_vertices, np.float64)[0]          # [F,3,3]
    F = fv.shape[0]
    x = fv[:, :, 0]; y = fv[:, :, 1]; z = fv[:, :, 2]
    x0, x1, x2 = x[:, 0], x[:, 1], x[:, 2]
    y0, y1, y2 = y[:, 0], y[:, 1], y[:, 2]

    den = (y1 - y2) * (x0 - x2) + (x2 - x1) * (y0 - y2)
    den = np.where(np.abs(den) < 1e-10, 1e-10, den)
    W0c = np.stack([(-(y1 - y2) * x2 - (x2 - x1) * y2) / den,
                    (y1 - y2) / den, (x2 - x1) / den, np.zeros(F)], -1)
    W1c = np.stack([(-(y2 - y0) * x2 - (x0 - x2) * y2) / den,
                    (y2 - y0) / den, (x0 - x2) / den, np.zeros(F)], -1)

    anchors = [(x0, y0), (x1, y1), (x2, y2)]
    pairs = [(0, 1), (1, 2), (2, 0)]
    # per edge: U = ((p-a).d)/|d| (along-line coord), LD = cross(p-a, d)/|d|
    # (signed line distance). d2seg = LD^2 + max(|U - L/2| - L/2, 0)^2.
    UT = np.zeros((3, F, 4)); S2 = np.zeros((3, F, 4)); HL = np.zeros((3, F))
    for e, (ia, ib) in enumerate(pairs):
        ax, ay = anchors[ia]; bx, by = anchors[ib]
        dx, dy = bx - ax, by - ay
        L = np.sqrt(np.maximum(dx * dx + dy * dy, 1e-12))
        iL = 1.0 / L
        UT[e, :, 0] = (-ax * dx - ay * dy) * iL
        UT[e, :, 1] = dx * iL
        UT[e, :, 2] = dy * iL
        S2[e, :, 0] = (ay * dx - ax * dy) * iL
        S2[e, :, 1] = dy * iL
        S2[e, :, 2] = -dx * iL
        HL[e] = L / 2.0
    iz = 1.0 / z
    zmin = z.min(1); zmax = z.max(1)
    assert z.min() > NEAR + 0.05 and z.max() < FAR - 0.05, \
        "kernel fast path assumes all vertex depths strictly inside (NEAR,FAR)"
    return dict(W0c=W0c, W1c=W1c, UT=UT, S2=S2, HL=HL, iz=iz,
                ymin=y.min(1), ymax=y.max(1), xmin=x.min(1), xmax=x.max(1),
                zmin=zmin, zmax=zmax)


def _cull_and_balance(prep):
    """Per tile (4 rows x 128 px), the kept-face list; balanced so all cores
    share one chunk-count pattern. Returns (pattern, assign) where
    assign[core][pos] = (tile_index, face_index_array padded with -1)."""
    nyb = H // 4
    pixc = ((np.arange(H) + 0.5) / H) * 2.0 - 1.0
    tiles = []
    for yb in range(nyb):
        for xb in range(2):
            tiles.append((pixc[4 * yb], pixc[4 * yb + 3],
                          pixc[128 * xb], pixc[128 * xb + 127]))
    tiles = np.array(tiles)                                # [nb, 4]
    nb = len(tiles)
    ygap = np.maximum(0.0, np.maximum(
        prep['ymin'][None, :] - tiles[:, 1:2],
        tiles[:, 0:1] - prep['ymax'][None, :]))
    xgap = np.maximum(0.0, np.maximum(
        prep['xmin'][None, :] - tiles[:, 3:4],
        tiles[:, 2:3] - prep['xmax'][None, :]))
    gap = np.sqrt(xgap ** 2 + ygap ** 2)
    znUB = (FAR - prep['zmin']) / (FAR - NEAR)
    znLB = (FAR - prep['zmax']) / (FAR - NEAR)
    D = znLB.max()
    MH = znUB.max()
    # every pixel's true zmax lies in [D, MH]; if that window is narrow a
    # single global softmax shift MH is exact (no over/underflow possible)
    assert MH - D <= 0.07, "global-shift fast path needs a narrow zmax window"
    keep = (gap < 0.17) | ((gap / SIGMA) + (D - znUB) / GAMMA < 87.0)  # [nb,F]

    counts = np.maximum(1, np.ceil(keep.sum(1) / 128).astype(int))
    order = np.argsort(-counts, kind='stable')             # bands, desc count
    pattern = [int(counts[order[p * NCORES]]) for p in range(NPOS)]
    assign = [[None] * NPOS for _ in range(NCORES)]
    for p in range(NPOS):
        for c in range(NCORES):
            b = int(order[p * NCORES + c])
            faces = np.nonzero(keep[b])[0]
            pad = pattern[p] * 128 - len(faces)
            assert pad >= 0
            faces = np.concatenate([faces, -np.ones(pad, np.int64)])
            assign[c][p] = (b, faces)
    return pattern, assign, float(max(MH, EPS))


# 3-way bf16 split: x = h + m + l with each part bf16-exact. Products of
# bf16-exact values are exact in the PE's f32r mode, so a 6-combo expansion
# (dropping <1e-7 cross terms) gives fp32-class precision at full PE rate.
COMBOS = [(0, 0), (0, 1), (1, 0), (0, 2), (1, 1), (2, 0)]
NK = 4 * len(COMBOS)


def _split3(a):
    a = np.asarray(a, np.float64)
    h = a.astype(ml_dtypes.bfloat16).astype(np.float64)
    r = a - h
    m = r.astype(ml_dtypes.bfloat16).astype(np.float64)
    l = (r - m).astype(ml_dtypes.bfloat16).astype(np.float64)
    return [h, m, l]


def _face_arrays(prep, textures, faces):
    """Pack per-chunk coefficient/texture/scalar arrays for one chunk of 128
    face slots (index -1 = inert dummy)."""
    f = np.asarray(faces)
    dummy = f < 0
    fi = np.where(dummy, 0, f)

    def D(a):  # zero out dummies
        a = np.asarray(a, np.float64).copy()
        a[dummy] = 0.0
        return a

    # quantity order: U01,LD01,U12,LD12,U20,LD20,W0,W1,W2 -> coef[4, 9, 128]
    coef = np.zeros((4, 9, 128))
    for e in range(3):
        coef[:, 2 * e, :] = D(prep['UT'][e][fi]).T
        coef[:, 2 * e + 1, :] = D(prep['S2'][e][fi]).T
    coef[:, 6, :] = D(prep['W0c'][fi]).T
    coef[:, 7, :] = D(prep['W1c'][fi]).T
    # dummies: W0=W1=-1 (outside, wc2=1), LD=10 (dist 10 -> prob 0),
    # iz=0.011 -> zp~90.9 -> zn~0.092 (never the argmax), halfL=0.5
    coef[0, 1, dummy] = 10.0
    coef[0, 3, dummy] = 10.0
    coef[0, 5, dummy] = 10.0
    coef[0, 6, dummy] = -1.0
    coef[0, 7, dummy] = -1.0
    coef[:, 8, :] = -coef[:, 6, :] - coef[:, 7, :]
    coef[0, 8, :] += 1.0                                   # w2 = 1 - w0 - w1
    cs = _split3(coef)
    coefk = np.zeros((NK, 9, 128), np.float32)
    for t, (ci, bi) in enumerate(COMBOS):
        coefk[4 * t:4 * t + 4] = cs[ci].astype(np.float32)

    tex = np.asarray(textures, np.float64)[0][fi]          # [128,3,3] (k,c)
    tex[dummy] = 0.0

    scal = np.zeros((128, 9))
    izf = prep['iz'][fi]
    izf[dummy] = 0.011
    scal[:, 0:3] = izf
    hlf = prep['HL'][:, fi].T
    hlf[dummy] = 0.5
    scal[:, 3:6] = hlf
    scal[:, 6:9] = -hlf
    return coefk, tex, scal


def _build_program(pattern, mhat):
    """One SPMD Bass program; chunk counts per position given by pattern."""
    totc = sum(pattern)
    kmax = max(pattern)
    nc = bacc.Bacc("TRN2", target_bir_lowering=False, debug=False,
                   num_devices=NCORES)
    d_coef = nc.dram_tensor("coef", [totc, 24, 9 * 128], F32R, kind="ExternalInput")
    d_basis = nc.dram_tensor("basis", [NPOS, 24, TP], F32R, kind="ExternalInput")
    d_tex = nc.dram_tensor("tex", [128, totc * 9], FP, kind="ExternalInput")
    d_scal = nc.dram_tensor("scal", [128, totc * 9], FP, kind="ExternalInput")
    d_out = nc.dram_tensor("out6", [5, NPOS * TP], FP, kind="ExternalOutput")

    with ExitStack() as ctx:
        tc = ctx.enter_context(tile.TileContext(nc))
        const = ctx.enter_context(tc.tile_pool(name="const", bufs=1))
        stage = ctx.enter_context(tc.tile_pool(name="stage", bufs=3))
        basp = ctx.enter_context(tc.tile_pool(name="basp", bufs=3))
        work = ctx.enter_context(tc.tile_pool(name="work", bufs=2))
        store = ctx.enter_context(tc.tile_pool(name="store", bufs=3))
        zm = ctx.enter_context(tc.tile_pool(name="zm", bufs=3))
        qp = ctx.enter_context(tc.tile_pool(name="qp", bufs=6, space="PSUM"))
        accp = ctx.enter_context(tc.tile_pool(name="accp", bufs=2, space="PSUM"))

        onesc = const.tile([128, 1], BF)
        nc.vector.memset(onesc, 1.0)
        onesf = const.tile([128, 1], FP)
        nc.vector.memset(onesf, 1.0)
        b_sqrt = const.tile([128, 1], FP)
        nc.vector.memset(b_sqrt, 1e-12)
        b_ln = const.tile([128, 1], FP)
        nc.vector.memset(b_ln, 1e-30)
        b_exp = const.tile([128, 1], FP)
        nc.vector.memset(b_exp, -mhat / GAMMA)
        tex_sb = const.tile([128, totc * 9], FP)
        nc.sync.dma_start(out=tex_sb, in_=d_tex[:, :])
        scal_sb = const.tile([128, totc * 9], FP)
        nc.sync.dma_start(out=scal_sb, in_=d_scal[:, :])

        jj = 0
        for pos in range(NPOS):
            K = pattern[pos]
            bas = basp.tile([24, TP], F32R, tag="bas")
            nc.sync.dma_start(out=bas, in_=d_basis[pos, :, :])

            zn_st = store.tile([128, kmax, TP], FP, tag="zn_st")
            pv_st = store.tile([128, kmax, TP], FP, tag="pv_st")
            wn_st = [store.tile([128, kmax, TP], FP, tag=f"wn{k}_st",
                                 name=f"wn{k}_st") for k in range(3)]
            d2_st = store.tile([128, kmax, TP], FP, tag="d2_st")
            sb_st = store.tile([128, kmax, TP], mybir.dt.uint32, tag="sb_st")
            acc = accp.tile([65, TP], FP, tag="acc")

            # ---- phase 1a: per chunk, everything except sqrt/sigmoid/ln ----
            for j in range(K):
                cj = jj + j
                st = stage.tile([24, 9 * 128], F32R, tag="st")
                nc.sync.dma_start(out=st, in_=d_coef[cj, :, :])
                q = [qp.tile([128, TP], FP, tag="q", name=f"q{qi}")
                     for qi in range(9)]
                for qi in range(9):
                    nc.tensor.matmul(q[qi], st[:, qi * 128:(qi + 1) * 128],
                                     bas, start=True, stop=True)
                sc = lambda i: scal_sb[:, cj * 9 + i: cj * 9 + i + 1]

                # d2_e = LD^2 + max(|U - L/2| - L/2, 0)^2
                ru = [work.tile([128, TP], FP, tag=f"ru{e}", name=f"ru{e}")
                      for e in range(3)]
                tt = [work.tile([128, TP], FP, tag=f"t{e}", name=f"t{e}")
                      for e in range(3)]
                for e in range(3):
                    nc.scalar.activation(ru[e], q[2 * e + 1], AF.Square)
                    nc.scalar.activation(tt[e], q[2 * e], AF.Identity,
                                         bias=sc(6 + e))        # U - L/2
                    nc.vector.tensor_scalar(
                        out=tt[e].bitcast(mybir.dt.uint32),
                        in0=tt[e].bitcast(mybir.dt.uint32),
                        scalar1=0x7fffffff, scalar2=None,
                        op0=AL.bitwise_and)                     # abs
                    nc.scalar.activation(tt[e], tt[e], AF.Relu,
                                         bias=sc(6 + e))        # overshoot
                    nc.scalar.activation(tt[e], tt[e], AF.Square)
                    nc.vector.tensor_tensor(out=ru[e], in0=ru[e], in1=tt[e],
                                            op=AL.add)
                nc.vector.tensor_tensor(out=ru[0], in0=ru[0], in1=ru[1],
                                        op=AL.min)
                nc.vector.tensor_tensor(out=d2_st[:, j, :], in0=ru[0],
                                        in1=ru[2], op=AL.min)

                cw0 = work.tile([128, TP], FP, tag="cw0")
                nc.scalar.activation(cw0, q[6], AF.Copy)
                cw1 = work.tile([128, TP], FP, tag="cw1")
                nc.scalar.activation(cw1, q[7], AF.Copy)
                cw2 = work.tile([128, TP], FP, tag="cw2")
                nc.scalar.activation(cw2, q[8], AF.Copy)
                m1 = work.tile([128, TP], FP, tag="m1")
                nc.vector.tensor_tensor(out=m1, in0=cw0, in1=cw1, op=AL.min)
                nc.vector.tensor_tensor(out=m1, in0=m1, in1=cw2, op=AL.min)
                nc.vector.tensor_scalar(out=sb_st[:, j, :],
                                        in0=m1.bitcast(mybir.dt.uint32),
                                        scalar1=0x80000000, scalar2=None,
                                        op0=AL.bitwise_and)

                wc0, wc1, wc2 = cw0, cw1, cw2
                nc.vector.tensor_scalar(out=wc0, in0=cw0, scalar1=0.0,
                                        scalar2=1.0, op0=AL.max, op1=AL.min)
                nc.vector.tensor_scalar(out=wc1, in0=cw1, scalar1=0.0,
                                        scalar2=1.0, op0=AL.max, op1=AL.min)
                nc.vector.tensor_scalar(out=wc2, in0=cw2, scalar1=0.0,
                                        scalar2=1.0, op0=AL.max, op1=AL.min)
                s01 = work.tile([128, TP], FP, tag="s01")
                nc.vector.tensor_tensor(out=s01, in0=wc0, in1=wc1, op=AL.add)
                nc.vector.tensor_tensor(out=s01, in0=s01, in1=wc2, op=AL.add)
                invs = work.tile([128, TP], FP, tag="invs")
                nc.vector.reciprocal_approx_fast(out=invs, in_=s01)
                r1 = work.tile([128, TP], FP, tag="r1")
                nc.vector.tensor_scalar(out=r1, in0=wc0, scalar1=sc(0),
                                        scalar2=None, op0=AL.mult)
                nc.vector.scalar_tensor_tensor(out=r1, in0=wc1, scalar=sc(1),
                                               in1=r1, op0=AL.mult, op1=AL.add)
                nc.vector.scalar_tensor_tensor(out=r1, in0=wc2, scalar=sc(2),
                                               in1=r1, op0=AL.mult, op1=AL.add)
                nc.vector.tensor_tensor(out=r1, in0=r1, in1=invs, op=AL.mult)
                nc.vector.reciprocal_approx_fast(out=s01, in_=r1)   # zp
                nc.vector.tensor_scalar(out=zn_st[:, j, :], in0=s01,
                                        scalar1=-1.0 / (FAR - NEAR),
                                        scalar2=FAR / (FAR - NEAR),
                                        op0=AL.mult, op1=AL.add)
                for k, wck in enumerate([wc0, wc1, wc2]):
                    nc.gpsimd.tensor_tensor(out=wn_st[k][:, j, :], in0=wck,
                                            in1=invs, op=AL.mult)

            # ---- phase 1b: batched sqrt / sign / sigmoid / ln ----
            for j in range(K):
                nc.scalar.activation(d2_st[:, j, :], d2_st[:, j, :], AF.Sqrt,
                                     bias=b_sqrt)
            for j in range(K):
                nc.vector.tensor_tensor(
                    out=d2_st[:, j, :].bitcast(mybir.dt.uint32),
                    in0=d2_st[:, j, :].bitcast(mybir.dt.uint32),
                    in1=sb_st[:, j, :], op=AL.bitwise_or)
            for j in range(K):
                nc.scalar.activation(pv_st[:, j, :], d2_st[:, j, :],
                                     AF.Sigmoid, scale=1.0 / SIGMA)
            for j in range(K):
                q1m = work.tile([128, TP], FP, tag="q1m")
                nc.vector.tensor_scalar(out=q1m, in0=pv_st[:, j, :],
                                        scalar1=-1.0, scalar2=1.0,
                                        op0=AL.mult, op1=AL.add)
                lq = work.tile([128, TP], FP, tag="lq")
                nc.scalar.activation(lq, q1m, AF.Ln, bias=b_ln)
                nc.tensor.matmul(acc[64:65, :], onesf[:, 0:1], lq,
                                 start=(j == 0), stop=(j == K - 1))

            # ---- phase 3: exp weights (global shift), rgb/dsum accum ----
            for j in range(K):
                cj = jj + j
                d = work.tile([128, TP], FP, tag="d")
                nc.scalar.activation(d, zn_st[:, j, :], AF.Exp,
                                     scale=1.0 / GAMMA, bias=b_exp)
                nc.vector.tensor_tensor(out=d, in0=pv_st[:, j, :], in1=d,
                                        op=AL.mult)             # wexp
                for k in range(3):
                    g = work.tile([128, TP], FP, tag="g", bufs=3)
                    eng = nc.vector if k == 0 else nc.gpsimd
                    eng.tensor_tensor(out=g, in0=d,
                                      in1=wn_st[k][:, j, :], op=AL.mult)
                    nc.tensor.matmul(
                        acc[0:3, :],
                        tex_sb[:, cj * 9 + k * 3: cj * 9 + (k + 1) * 3], g,
                        start=(j == 0 and k == 0),
                        stop=(j == K - 1 and k == 2))
                nc.tensor.matmul(acc[32:33, :], onesf[:, 0:1], d,
                                 start=(j == 0), stop=(j == K - 1))

            o6 = zm.tile([65, TP], FP, tag="o6")
            nc.scalar.activation(o6[0:3, :], acc[0:3, :], AF.Copy)
            nc.scalar.activation(o6[32:33, :], acc[32:33, :], AF.Copy)
            nc.scalar.activation(o6[64:65, :], acc[64:65, :], AF.Copy)
            nc.sync.dma_start(out=d_out[0:3, pos * TP:(pos + 1) * TP],
                              in_=o6[0:3, :])
            nc.sync.dma_start(out=d_out[3:4, pos * TP:(pos + 1) * TP],
                              in_=o6[32:33, :])
            nc.sync.dma_start(out=d_out[4:5, pos * TP:(pos + 1) * TP],
                              in_=o6[64:65, :])
            jj += K
    nc.compile()
    return nc


def kernel(face_vertices, face_textures):
    prep = _host_prep(face_vertices)
    pattern, assign, mhat = _cull_and_balance(prep)
    totc = sum(pattern)

    pix = ((np.arange(H, dtype=np.float64) + 0.5) / H) * 2.0 - 1.0
    in_maps = []
    for c in range(NCORES):
        coef = np.zeros((totc, NK, 9 * 128), np.float32)
        tex = np.zeros((128, totc * 9), np.float32)
        scal = np.zeros((128, totc * 9), np.float32)
        basis = np.zeros((NPOS, NK, TP), np.float32)
        jj = 0
        for pos in range(NPOS):
            b, faces = assign[c][pos]
            yb, xb = b // 2, b % 2
            py = np.repeat(pix[4 * yb:4 * yb + 4], 128)
            px = np.tile(pix[128 * xb:128 * xb + 128], 4)
            b4 = np.stack([np.ones(TP), px, py, px ** 2 + py ** 2])
            bs = _split3(b4)
            for t, (ci, bi) in enumerate(COMBOS):
                basis[pos, 4 * t:4 * t + 4] = bs[bi].astype(np.float32)
            for j in range(pattern[pos]):
                cf, tx, sl = _face_arrays(prep, face_textures,
                                          faces[j * 128:(j + 1) * 128])
                coef[jj] = cf.reshape(NK, 9 * 128)
                tex[:, jj * 9:(jj + 1) * 9] = tx.reshape(128, 9)
                scal[:, jj * 9:(jj + 1) * 9] = sl
                jj += 1
        in_maps.append({"coef": coef, "basis": basis, "tex": tex, "scal": scal})

    nc = _build_program(pattern, mhat)
    global LAST_RESULT
    if TRACE:
        res = run_bass_kernel_spmd(nc, in_maps, core_ids=list(range(NCORES)),
                                   trace=True)
    else:
        res = run_bass_kernel_spmd(nc, in_maps, core_ids=list(range(NCORES)))
    LAST_RESULT = res

    out = np.zeros((1, 4, H, W), np.float32)
    for c in range(NCORES):
        o6 = res.results[c]["out6"]                        # [6, NPOS*TP]
        for pos in range(NPOS):
            b, _ = assign[c][pos]
            yb, xb = b // 2, b % 2
            seg = o6[:, pos * TP:(pos + 1) * TP]
            wbg = np.float32(np.exp((EPS - mhat) / GAMMA))
            dsum = seg[3] + wbg
            rgb = seg[0:3] / dsum[None]
            alpha = 1.0 - np.exp(seg[4])
            ys = slice(4 * yb, 4 * yb + 4)
            xs = slice(128 * xb, 128 * xb + 128)
            out[0, 0:3, ys, xs] = rgb.reshape(3, 4, 128)
            out[0, 3, ys, xs] = alpha.reshape(4, 128)
    return out


# revision 24
# speedup vs baseline: 1.8452x; 1.0135x over previous
"""SoftRas-style soft rasterizer on 8 Trainium2 NeuronCores.

Strategy:
- All per-(face,pixel) affine quantities (barycentric w0/w1, edge projections
  u/l2, squared vertex distances) are produced by TensorE matmuls against the
  pixel basis [1, px, py, px^2+py^2] (K=4).
- The nonlinear chain (clip/sqrt/sigmoid/reciprocal/exp/log) runs on
  VectorE/ScalarE with faces on partitions (128/chunk), pixels on the free dim.
- Per-pixel max over faces (softmax shift) via PE transposes + free-dim max.
- Face-direction reductions (rgb accumulation, dsum, sum log(1-p)) via PE
  matmuls against textures / ones.
- Host (numpy): per-face coefficient prep, per-tile face culling, per-core
  load balancing (every core gets an identical chunk-count pattern so one
  SPMD program serves all 8 cores), final divide + alpha exponentiation.
"""
import sys
sys.path.insert(0, '/opt/trn_rl_repo')
import numpy as np
import ml_dtypes
from contextlib import ExitStack

import concourse.bass as bass
import concourse.bacc as bacc
import concourse.tile as tile
import concourse.mybir as mybir
from concourse.bass_utils import run_bass_kernel_spmd
from concourse.masks import make_identity

TRACE = False
LAST_RESULT = None

F_TOT = 512
H = W = 256
NCORES = 8
TP = 512              # pixels per tile position (2 image rows)
NPOS = (H * W) // (NCORES * TP)   # 16 tile positions per core
SIGMA = 1e-2
GAMMA = 1e-3
EPS = 1e-3
NEAR, FAR = 1.0, 100.0
FP = mybir.dt.float32
F32R = mybir.dt.float32r
BF = mybir.dt.bfloat16
AL = mybir.AluOpType
AF = mybir.ActivationFunctionType


def _host_prep(face_vertices):
    """Per-face coefficients in basis [1, px, py, r2], fp64 -> fp32."""
    fv = np.asarray(face_vertices, np.float64)[0]          # [F,3,3]
    F = fv.shape[0]
    x = fv[:, :, 0]; y = fv[:, :, 1]; z = fv[:, :, 2]
    x0, x1, x2 = x[:, 0], x[:, 1], x[:, 2]
    y0, y1, y2 = y[:, 0], y[:, 1], y[:, 2]

    den = (y1 - y2) * (x0 - x2) + (x2 - x1) * (y0 - y2)
    den = np.where(np.abs(den) < 1e-10, 1e-10, den)
    W0c = np.stack([(-(y1 - y2) * x2 - (x2 - x1) * y2) / den,
                    (y1 - y2) / den, (x2 - x1) / den, np.zeros(F)], -1)
    W1c = np.stack([(-(y2 - y0) * x2 - (x0 - x2) * y2) / den,
                    (y2 - y0) / den, (x0 - x2) / den, np.zeros(F)], -1)

    anchors = [(x0, y0), (x1, y1), (x2, y2)]
    pairs = [(0, 1), (1, 2), (2, 0)]
    # per edge: U = ((p-a).d)/|d| (along-line coord), LD = cross(p-a, d)/|d|
    # (signed line distance). d2seg = LD^2 + max(|U - L/2| - L/2, 0)^2.
    UT = np.zeros((3, F, 4)); S2 = np.zeros((3, F, 4)); HL = np.zeros((3, F))
    for e, (ia, ib) in enumerate(pairs):
        ax, ay = anchors[ia]; bx, by = anchors[ib]
        dx, dy = bx - ax, by - ay
        L = np.sqrt(np.maximum(dx * dx + dy * dy, 1e-12))
        iL = 1.0 / L
        UT[e, :, 0] = (-ax * dx - ay * dy) * iL
        UT[e, :, 1] = dx * iL
        UT[e, :, 2] = dy * iL
        S2[e, :, 0] = (ay * dx - ax * dy) * iL
        S2[e, :, 1] = dy * iL
        S2[e, :, 2] = -dx * iL
        HL[e] = L / 2.0
    iz = 1.0 / z
    zmin = z.min(1); zmax = z.max(1)
    assert z.min() > NEAR + 0.05 and z.max() < FAR - 0.05, \
        "kernel fast path assumes all vertex depths strictly inside (NEAR,FAR)"
    return dict(W0c=W0c, W1c=W1c, UT=UT, S2=S2, HL=HL, iz=iz,
                ymin=y.min(1), ymax=y.max(1), xmin=x.min(1), xmax=x.max(1),
                zmin=zmin, zmax=zmax)


def _cull_and_balance(prep):
    """Per tile (4 rows x 128 px), the kept-face list; balanced so all cores
    share one chunk-count pattern. Returns (pattern, assign) where
    assign[core][pos] = (tile_index, face_index_array padded with -1)."""
    nyb = H // 4
    pixc = ((np.arange(H) + 0.5) / H) * 2.0 - 1.0
    tiles = []
    for yb in range(nyb):
        for xb in range(2):
            tiles.append((pixc[4 * yb], pixc[4 * yb + 3],
                          pixc[128 * xb], pixc[128 * xb + 127]))
    tiles = np.array(tiles)                                # [nb, 4]
    nb = len(tiles)
    ygap = np.maximum(0.0, np.maximum(
        prep['ymin'][None, :] - tiles[:, 1:2],
        tiles[:, 0:1] - prep['ymax'][None, :]))
    xgap = np.maximum(0.0, np.maximum(
        prep['xmin'][None, :] - tiles[:, 3:4],
        tiles[:, 2:3] - prep['xmax'][None, :]))
    gap = np.sqrt(xgap ** 2 + ygap ** 2)
    znUB = (FAR - prep['zmin']) / (FAR - NEAR)
    znLB = (FAR - prep['zmax']) / (FAR - NEAR)
    D = znLB.max()
    MH = znUB.max()
    # every pixel's true zmax lies in [D, MH]; if that window is narrow a
    # single global softmax shift MH is exact (no over/underflow possible)
    assert MH - D <= 0.07, "global-shift fast path needs a narrow zmax window"
    keep = (gap < 0.17) | ((gap / SIGMA) + (D - znUB) / GAMMA < 87.0)  # [nb,F]

    counts = np.maximum(1, np.ceil(keep.sum(1) / 128).astype(int))
    order = np.argsort(-counts, kind='stable')             # bands, desc count
    pattern = [int(counts[order[p * NCORES]]) for p in range(NPOS)]
    assign = [[None] * NPOS for _ in range(NCORES)]
    for p in range(NPOS):
        for c in range(NCORES):
            b = int(order[p * NCORES + c])
            faces = np.nonzero(keep[b])[0]
            pad = pattern[p] * 128 - len(faces)
            assert pad >= 0
            faces = np.concatenate([faces, -np.ones(pad, np.int64)])
            assign[c][p] = (b, faces)
    return pattern, assign, float(max(MH, EPS))


# 3-way bf16 split: x = h + m + l with each part bf16-exact. Products of
# bf16-exact values are exact in the PE's f32r mode, so a 6-combo expansion
# (dropping <1e-7 cross terms) gives fp32-class precision at full PE rate.
COMBOS = [(0, 0), (0, 1), (1, 0), (0, 2), (1, 1), (2, 0)]
NK = 4 * len(COMBOS)


def _split3(a):
    a = np.asarray(a, np.float64)
    h = a.astype(ml_dtypes.bfloat16).astype(np.float64)
    r = a - h
    m = r.astype(ml_dtypes.bfloat16).astype(np.float64)
    l = (r - m).astype(ml_dtypes.bfloat16).astype(np.float64)
    return [h, m, l]


def _face_arrays(prep, textures, faces):
    """Pack per-chunk coefficient/texture/scalar arrays for one chunk of 128
    face slots (index -1 = inert dummy)."""
    f = np.asarray(faces)
    dummy = f < 0
    fi = np.where(dummy, 0, f)

    def D(a):  # zero out dummies
        a = np.asarray(a, np.float64).copy()
        a[dummy] = 0.0
        return a

    # quantity order: U01,LD01,U12,LD12,U20,LD20,W0,W1,W2 -> coef[4, 9, 128]
    coef = np.zeros((4, 9, 128))
    for e in range(3):
        coef[:, 2 * e, :] = D(prep['UT'][e][fi]).T
        coef[:, 2 * e + 1, :] = D(prep['S2'][e][fi]).T
    coef[:, 6, :] = D(prep['W0c'][fi]).T
    coef[:, 7, :] = D(prep['W1c'][fi]).T
    # dummies: W0=W1=-1 (outside, wc2=1), LD=10 (dist 10 -> prob 0),
    # iz=0.011 -> zp~90.9 -> zn~0.092 (never the argmax), halfL=0.5
    coef[0, 1, dummy] = 10.0
    coef[0, 3, dummy] = 10.0
    coef[0, 5, dummy] = 10.0
    coef[0, 6, dummy] = -1.0
    coef[0, 7, dummy] = -1.0
    coef[:, 8, :] = -coef[:, 6, :] - coef[:, 7, :]
    coef[0, 8, :] += 1.0                                   # w2 = 1 - w0 - w1
    cs = _split3(coef)
    coefk = np.zeros((NK, 9, 128), np.float32)
    for t, (ci, bi) in enumerate(COMBOS):
        coefk[4 * t:4 * t + 4] = cs[ci].astype(np.float32)

    tex = np.asarray(textures, np.float64)[0][fi]          # [128,3,3] (k,c)
    tex[dummy] = 0.0

    scal = np.zeros((128, 9))
    izf = prep['iz'][fi]
    izf[dummy] = 0.011
    scal[:, 0:3] = izf
    hlf = prep['HL'][:, fi].T
    hlf[dummy] = 0.5
    scal[:, 3:6] = hlf
    scal[:, 6:9] = -hlf
    return coefk, tex, scal


def _build_program(pattern, mhat):
    """One SPMD Bass program; chunk counts per position given by pattern."""
    totc = sum(pattern)
    kmax = max(pattern)
    nc = bacc.Bacc("TRN2", target_bir_lowering=False, debug=False,
                   num_devices=NCORES)
    d_coef = nc.dram_tensor("coef", [totc, 24, 9 * 128], F32R, kind="ExternalInput")
    d_basis = nc.dram_tensor("basis", [NPOS, 24, TP], F32R, kind="ExternalInput")
    d_tex = nc.dram_tensor("tex", [128, totc * 9], FP, kind="ExternalInput")
    d_scal = nc.dram_tensor("scal", [128, totc * 9], FP, kind="ExternalInput")
    d_out = nc.dram_tensor("out6", [5, NPOS * TP], FP, kind="ExternalOutput")

    with ExitStack() as ctx:
        tc = ctx.enter_context(tile.TileContext(nc))
        const = ctx.enter_context(tc.tile_pool(name="const", bufs=1))
        stage = ctx.enter_context(tc.tile_pool(name="stage", bufs=3))
        basp = ctx.enter_context(tc.tile_pool(name="basp", bufs=3))
        work = ctx.enter_context(tc.tile_pool(name="work", bufs=2))
        store = ctx.enter_context(tc.tile_pool(name="store", bufs=3))
        zm = ctx.enter_context(tc.tile_pool(name="zm", bufs=3))
        qp = ctx.enter_context(tc.tile_pool(name="qp", bufs=6, space="PSUM"))
        accp = ctx.enter_context(tc.tile_pool(name="accp", bufs=2, space="PSUM"))

        onesc = const.tile([128, 1], BF)
        nc.vector.memset(onesc, 1.0)
        onesf = const.tile([128, 1], FP)
        nc.vector.memset(onesf, 1.0)
        b_sqrt = const.tile([128, 1], FP)
        nc.vector.memset(b_sqrt, 1e-12)
        b_ln = const.tile([128, 1], FP)
        nc.vector.memset(b_ln, 1e-30)
        b_exp = const.tile([128, 1], FP)
        nc.vector.memset(b_exp, -mhat / GAMMA)
        tex_sb = const.tile([128, totc * 9], FP)
        nc.sync.dma_start(out=tex_sb, in_=d_tex[:, :])
        scal_sb = const.tile([128, totc * 9], FP)
        nc.sync.dma_start(out=scal_sb, in_=d_scal[:, :])

        jj = 0
        for pos in range(NPOS):
            K = pattern[pos]
            bas = basp.tile([24, TP], F32R, tag="bas")
            nc.sync.dma_start(out=bas, in_=d_basis[pos, :, :])

            zn_st = store.tile([128, kmax, TP], FP, tag="zn_st")
            pv_st = store.tile([128, kmax, TP], FP, tag="pv_st")
            wn_st = [store.tile([128, kmax, TP], FP, tag=f"wn{k}_st",
                                 name=f"wn{k}_st") for k in range(3)]
            d2_st = store.tile([128, kmax, TP], FP, tag="d2_st")
            sb_st = store.tile([128, kmax, TP], mybir.dt.uint32, tag="sb_st")
            acc = accp.tile([65, TP], FP, tag="acc")

            # ---- phase 1a: per chunk, everything except sqrt/sigmoid/ln ----
            for j in range(K):
                cj = jj + j
                st = stage.tile([24, 9 * 128], F32R, tag="st")
                nc.sync.dma_start(out=st, in_=d_coef[cj, :, :])
                q = [qp.tile([128, TP], FP, tag="q", name=f"q{qi}")
                     for qi in range(9)]
                for qi in range(9):
                    nc.tensor.matmul(q[qi], st[:, qi * 128:(qi + 1) * 128],
                                     bas, start=True, stop=True)
                sc = lambda i: scal_sb[:, cj * 9 + i: cj * 9 + i + 1]

                # d2_e = LD^2 + max(|U - L/2| - L/2, 0)^2
                ru = [work.tile([128, TP], FP, tag=f"ru{e}", name=f"ru{e}")
                      for e in range(3)]
                tt = [work.tile([128, TP], FP, tag=f"t{e}", name=f"t{e}")
                      for e in range(3)]
                for e in range(3):
                    nc.scalar.activation(ru[e], q[2 * e + 1], AF.Square)
                    nc.scalar.activation(tt[e], q[2 * e], AF.Identity,
                                         bias=sc(6 + e))        # U - L/2
                    nc.vector.tensor_scalar(
                        out=tt[e].bitcast(mybir.dt.uint32),
                        in0=tt[e].bitcast(mybir.dt.uint32),
                        scalar1=0x7fffffff, scalar2=None,
                        op0=AL.bitwise_and)                     # abs
                    nc.scalar.activation(tt[e], tt[e], AF.Relu,
                                         bias=sc(6 + e))        # overshoot
                    nc.scalar.activation(tt[e], tt[e], AF.Square)
                    nc.vector.tensor_tensor(out=ru[e], in0=ru[e], in1=tt[e],
                                            op=AL.add)
                nc.vector.tensor_tensor(out=ru[0], in0=ru[0], in1=ru[1],
                                        op=AL.min)
                nc.vector.tensor_tensor(out=d2_st[:, j, :], in0=ru[0],
                                        in1=ru[2], op=AL.min)

                cw0 = work.tile([128, TP], FP, tag="cw0")
                nc.scalar.activation(cw0, q[6], AF.Copy)
                cw1 = work.tile([128, TP], FP, tag="cw1")
                nc.scalar.activation(cw1, q[7], AF.Copy)
                cw2 = work.tile([128, TP], FP, tag="cw2")
                nc.scalar.activation(cw2, q[8], AF.Copy)
                m1 = work.tile([128, TP], FP, tag="m1")
                nc.vector.tensor_tensor(out=m1, in0=cw0, in1=cw1, op=AL.min)
                nc.vector.tensor_tensor(out=m1, in0=m1, in1=cw2, op=AL.min)
                nc.vector.tensor_scalar(out=sb_st[:, j, :],
                                        in0=m1.bitcast(mybir.dt.uint32),
                                        scalar1=0x80000000, scalar2=None,
                                        op0=AL.bitwise_and)

                wc0, wc1, wc2 = cw0, cw1, cw2
                nc.vector.tensor_scalar(out=wc0, in0=cw0, scalar1=0.0,
                                        scalar2=1.0, op0=AL.max, op1=AL.min)
                nc.vector.tensor_scalar(out=wc1, in0=cw1, scalar1=0.0,
                                        scalar2=1.0, op0=AL.max, op1=AL.min)
                nc.vector.tensor_scalar(out=wc2, in0=cw2, scalar1=0.0,
                                        scalar2=1.0, op0=AL.max, op1=AL.min)
                s01 = work.tile([128, TP], FP, tag="s01")
                nc.vector.tensor_tensor(out=s01, in0=wc0, in1=wc1, op=AL.add)
                nc.vector.tensor_tensor(out=s01, in0=s01, in1=wc2, op=AL.add)
                invs = work.tile([128, TP], FP, tag="invs")
                nc.vector.reciprocal_approx_fast(out=invs, in_=s01)
                r1 = work.tile([128, TP], FP, tag="r1")
                nc.vector.tensor_scalar(out=r1, in0=wc0, scalar1=sc(0),
                                        scalar2=None, op0=AL.mult)
                nc.vector.scalar_tensor_tensor(out=r1, in0=wc1, scalar=sc(1),
                                               in1=r1, op0=AL.mult, op1=AL.add)
                nc.vector.scalar_tensor_tensor(out=r1, in0=wc2, scalar=sc(2),
                                               in1=r1, op0=AL.mult, op1=AL.add)
                nc.vector.tensor_tensor(out=r1, in0=r1, in1=invs, op=AL.mult)
                nc.vector.reciprocal_approx_fast(out=s01, in_=r1)   # zp
                nc.vector.tensor_scalar(out=zn_st[:, j, :], in0=s01,
                                        scalar1=-1.0 / (FAR - NEAR),
                                        scalar2=FAR / (FAR - NEAR),
                                        op0=AL.mult, op1=AL.add)
                for k, wck in enumerate([wc0, wc1, wc2]):
                    nc.gpsimd.tensor_tensor(out=wn_st[k][:, j, :], in0=wck,
                                            in1=invs, op=AL.mult)

            # ---- phase 1b: batched sqrt / sign / sigmoid / ln ----
            for j in range(K):
                nc.scalar.activation(d2_st[:, j, :], d2_st[:, j, :], AF.Sqrt,
                                     bias=b_sqrt)
            for j in range(K):
                nc.vector.tensor_tensor(
                    out=d2_st[:, j, :].bitcast(mybir.dt.uint32),
                    in0=d2_st[:, j, :].bitcast(mybir.dt.uint32),
                    in1=sb_st[:, j, :], op=AL.bitwise_or)
            for j in range(K):
                nc.scalar.activation(pv_st[:, j, :], d2_st[:, j, :],
                                     AF.Sigmoid, scale=1.0 / SIGMA)
            for j in range(K):
                q1m = work.tile([128, TP], FP, tag="q1m")
                nc.vector.tensor_scalar(out=q1m, in0=pv_st[:, j, :],
                                        scalar1=-1.0, scalar2=1.0,
                                        op0=AL.mult, op1=AL.add)
                lq = work.tile([128, TP], FP, tag="lq")
                nc.scalar.activation(lq, q1m, AF.Ln, bias=b_ln)
                nc.tensor.matmul(acc[64:65, :], onesf[:, 0:1], lq,
                                 start=(j == 0), stop=(j == K - 1))

            # ---- phase 3: exp weights (global shift), rgb/dsum accum ----
            for j in range(K):
                cj = jj + j
                d = work.tile([128, TP], FP, tag="d")
                nc.scalar.activation(d, zn_st[:, j, :], AF.Exp,
                                     scale=1.0 / GAMMA, bias=b_exp)
                nc.vector.tensor_tensor(out=d, in0=pv_st[:, j, :], in1=d,
                                        op=AL.mult)             # wexp
                for k in range(3):
                    g = work.tile([128, TP], FP, tag="g", bufs=3)
                    nc.gpsimd.tensor_tensor(out=g, in0=d,
                                            in1=wn_st[k][:, j, :], op=AL.mult)
                    nc.tensor.matmul(
                        acc[0:3, :],
                        tex_sb[:, cj * 9 + k * 3: cj * 9 + (k + 1) * 3], g,
                        start=(j == 0 and k == 0),
                        stop=(j == K - 1 and k == 2))
                nc.tensor.matmul(acc[32:33, :], onesf[:, 0:1], d,
                                 start=(j == 0), stop=(j == K - 1))

            o6 = zm.tile([65, TP], FP, tag="o6")
            nc.vector.tensor_copy(o6[0:3, :], acc[0:3, :])
            nc.scalar.activation(o6[32:33, :], acc[32:33, :], AF.Copy)
            nc.scalar.activation(o6[64:65, :], acc[64:65, :], AF.Copy)
            nc.sync.dma_start(out=d_out[0:3, pos * TP:(pos + 1) * TP],
                              in_=o6[0:3, :])
            nc.sync.dma_start(out=d_out[3:4, pos * TP:(pos + 1) * TP],
                              in_=o6[32:33, :])
            nc.sync.dma_start(out=d_out[4:5, pos * TP:(pos + 1) * TP],
                              in_=o6[64:65, :])
            jj += K
    nc.compile()
    return nc


def kernel(face_vertices, face_textures):
    prep = _host_prep(face_vertices)
    pattern, assign, mhat = _cull_and_balance(prep)
    totc = sum(pattern)

    pix = ((np.arange(H, dtype=np.float64) + 0.5) / H) * 2.0 - 1.0
    in_maps = []
    for c in range(NCORES):
        coef = np.zeros((totc, NK, 9 * 128), np.float32)
        tex = np.zeros((128, totc * 9), np.float32)
        scal = np.zeros((128, totc * 9), np.float32)
        basis = np.zeros((NPOS, NK, TP), np.float32)
        jj = 0
        for pos in range(NPOS):
            b, faces = assign[c][pos]
            yb, xb = b // 2, b % 2
            py = np.repeat(pix[4 * yb:4 * yb + 4], 128)
            px = np.tile(pix[128 * xb:128 * xb + 128], 4)
            b4 = np.stack([np.ones(TP), px, py, px ** 2 + py ** 2])
            bs = _split3(b4)
            for t, (ci, bi) in enumerate(COMBOS):
                basis[pos, 4 * t:4 * t + 4] = bs[bi].astype(np.float32)
            for j in range(pattern[pos]):
                cf, tx, sl = _face_arrays(prep, face_textures,
                                          faces[j * 128:(j + 1) * 128])
                coef[jj] = cf.reshape(NK, 9 * 128)
                tex[:, jj * 9:(jj + 1) * 9] = tx.reshape(128, 9)
                scal[:, jj * 9:(jj + 1) * 9] = sl
                jj += 1
        in_maps.append({"coef": coef, "basis": basis, "tex": tex, "scal": scal})

    nc = _build_program(pattern, mhat)
    global LAST_RESULT
    if TRACE:
        res = run_bass_kernel_spmd(nc, in_maps, core_ids=list(range(NCORES)),
                                   trace=True)
    else:
        res = run_bass_kernel_spmd(nc, in_maps, core_ids=list(range(NCORES)))
    LAST_RESULT = res

    out = np.zeros((1, 4, H, W), np.float32)
    for c in range(NCORES):
        o6 = res.results[c]["out6"]                        # [6, NPOS*TP]
        for pos in range(NPOS):
            b, _ = assign[c][pos]
            yb, xb = b // 2, b % 2
            seg = o6[:, pos * TP:(pos + 1) * TP]
            wbg = np.float32(np.exp((EPS - mhat) / GAMMA))
            dsum = seg[3] + wbg
            rgb = seg[0:3] / dsum[None]
            alpha = 1.0 - np.exp(seg[4])
            ys = slice(4 * yb, 4 * yb + 4)
            xs = slice(128 * xb, 128 * xb + 128)
            out[0, 0:3, ys, xs] = rgb.reshape(3, 4, 128)
            out[0, 3, ys, xs] = alpha.reshape(4, 128)
    return out


# revision 25
# speedup vs baseline: 1.8638x; 1.0101x over previous
"""SoftRas-style soft rasterizer on 8 Trainium2 NeuronCores.

Strategy:
- All per-(face,pixel) affine quantities (barycentric w0/w1, edge projections
  u/l2, squared vertex distances) are produced by TensorE matmuls against the
  pixel basis [1, px, py, px^2+py^2] (K=4).
- The nonlinear chain (clip/sqrt/sigmoid/reciprocal/exp/log) runs on
  VectorE/ScalarE with faces on partitions (128/chunk), pixels on the free dim.
- Per-pixel max over faces (softmax shift) via PE transposes + free-dim max.
- Face-direction reductions (rgb accumulation, dsum, sum log(1-p)) via PE
  matmuls against textures / ones.
- Host (numpy): per-face coefficient prep, per-tile face culling, per-core
  load balancing (every core gets an identical chunk-count pattern so one
  SPMD program serves all 8 cores), final divide + alpha exponentiation.
"""
import sys
sys.path.insert(0, '/opt/trn_rl_repo')
import numpy as np
import ml_dtypes
from contextlib import ExitStack

import concourse.bass as bass
import concourse.bacc as bacc
import concourse.tile as tile
import concourse.mybir as mybir
from concourse.bass_utils import run_bass_kernel_spmd
from concourse.masks import make_identity

TRACE = False
LAST_RESULT = None

F_TOT = 512
H = W = 256
NCORES = 8
TP = 512              # pixels per tile position (2 image rows)
NPOS = (H * W) // (NCORES * TP)   # 16 tile positions per core
SIGMA = 1e-2
GAMMA = 1e-3
EPS = 1e-3
NEAR, FAR = 1.0, 100.0
FP = mybir.dt.float32
F32R = mybir.dt.float32r
BF = mybir.dt.bfloat16
AL = mybir.AluOpType
AF = mybir.ActivationFunctionType


def _host_prep(face_vertices):
    """Per-face coefficients in basis [1, px, py, r2], fp64 -> fp32."""
    fv = np.asarray(face_vertices, np.float64)[0]          # [F,3,3]
    F = fv.shape[0]
    x = fv[:, :, 0]; y = fv[:, :, 1]; z = fv[:, :, 2]
    x0, x1, x2 = x[:, 0], x[:, 1], x[:, 2]
    y0, y1, y2 = y[:, 0], y[:, 1], y[:, 2]

    den = (y1 - y2) * (x0 - x2) + (x2 - x1) * (y0 - y2)
    den = np.where(np.abs(den) < 1e-10, 1e-10, den)
    W0c = np.stack([(-(y1 - y2) * x2 - (x2 - x1) * y2) / den,
                    (y1 - y2) / den, (x2 - x1) / den, np.zeros(F)], -1)
    W1c = np.stack([(-(y2 - y0) * x2 - (x0 - x2) * y2) / den,
                    (y2 - y0) / den, (x0 - x2) / den, np.zeros(F)], -1)

    anchors = [(x0, y0), (x1, y1), (x2, y2)]
    pairs = [(0, 1), (1, 2), (2, 0)]
    # per edge: U = ((p-a).d)/|d| (along-line coord), LD = cross(p-a, d)/|d|
    # (signed line distance). d2seg = LD^2 + max(|U - L/2| - L/2, 0)^2.
    UT = np.zeros((3, F, 4)); S2 = np.zeros((3, F, 4)); HL = np.zeros((3, F))
    for e, (ia, ib) in enumerate(pairs):
        ax, ay = anchors[ia]; bx, by = anchors[ib]
        dx, dy = bx - ax, by - ay
        L = np.sqrt(np.maximum(dx * dx + dy * dy, 1e-12))
        iL = 1.0 / L
        UT[e, :, 0] = (-ax * dx - ay * dy) * iL - L / 2.0   # y = U - L/2
        UT[e, :, 1] = dx * iL
        UT[e, :, 2] = dy * iL
        S2[e, :, 0] = (ay * dx - ax * dy) * iL
        S2[e, :, 1] = dy * iL
        S2[e, :, 2] = -dx * iL
        HL[e] = L / 2.0
    iz = 1.0 / z
    zmin = z.min(1); zmax = z.max(1)
    assert z.min() > NEAR + 0.05 and z.max() < FAR - 0.05, \
        "kernel fast path assumes all vertex depths strictly inside (NEAR,FAR)"
    return dict(W0c=W0c, W1c=W1c, UT=UT, S2=S2, HL=HL, iz=iz,
                ymin=y.min(1), ymax=y.max(1), xmin=x.min(1), xmax=x.max(1),
                zmin=zmin, zmax=zmax)


def _cull_and_balance(prep):
    """Per tile (4 rows x 128 px), the kept-face list; balanced so all cores
    share one chunk-count pattern. Returns (pattern, assign) where
    assign[core][pos] = (tile_index, face_index_array padded with -1)."""
    nyb = H // 4
    pixc = ((np.arange(H) + 0.5) / H) * 2.0 - 1.0
    tiles = []
    for yb in range(nyb):
        for xb in range(2):
            tiles.append((pixc[4 * yb], pixc[4 * yb + 3],
                          pixc[128 * xb], pixc[128 * xb + 127]))
    tiles = np.array(tiles)                                # [nb, 4]
    nb = len(tiles)
    ygap = np.maximum(0.0, np.maximum(
        prep['ymin'][None, :] - tiles[:, 1:2],
        tiles[:, 0:1] - prep['ymax'][None, :]))
    xgap = np.maximum(0.0, np.maximum(
        prep['xmin'][None, :] - tiles[:, 3:4],
        tiles[:, 2:3] - prep['xmax'][None, :]))
    gap = np.sqrt(xgap ** 2 + ygap ** 2)
    znUB = (FAR - prep['zmin']) / (FAR - NEAR)
    znLB = (FAR - prep['zmax']) / (FAR - NEAR)
    D = znLB.max()
    MH = znUB.max()
    # every pixel's true zmax lies in [D, MH]; if that window is narrow a
    # single global softmax shift MH is exact (no over/underflow possible)
    assert MH - D <= 0.07, "global-shift fast path needs a narrow zmax window"
    keep = (gap < 0.17) | ((gap / SIGMA) + (D - znUB) / GAMMA < 87.0)  # [nb,F]

    counts = np.maximum(1, np.ceil(keep.sum(1) / 128).astype(int))
    order = np.argsort(-counts, kind='stable')             # bands, desc count
    pattern = [int(counts[order[p * NCORES]]) for p in range(NPOS)]
    assign = [[None] * NPOS for _ in range(NCORES)]
    for p in range(NPOS):
        for c in range(NCORES):
            b = int(order[p * NCORES + c])
            faces = np.nonzero(keep[b])[0]
            pad = pattern[p] * 128 - len(faces)
            assert pad >= 0
            faces = np.concatenate([faces, -np.ones(pad, np.int64)])
            assign[c][p] = (b, faces)
    return pattern, assign, float(max(MH, EPS))


# 3-way bf16 split: x = h + m + l with each part bf16-exact. Products of
# bf16-exact values are exact in the PE's f32r mode, so a 6-combo expansion
# (dropping <1e-7 cross terms) gives fp32-class precision at full PE rate.
COMBOS = [(0, 0), (0, 1), (1, 0), (0, 2), (1, 1), (2, 0)]
NK = 4 * len(COMBOS)


def _split3(a):
    a = np.asarray(a, np.float64)
    h = a.astype(ml_dtypes.bfloat16).astype(np.float64)
    r = a - h
    m = r.astype(ml_dtypes.bfloat16).astype(np.float64)
    l = (r - m).astype(ml_dtypes.bfloat16).astype(np.float64)
    return [h, m, l]


def _face_arrays(prep, textures, faces):
    """Pack per-chunk coefficient/texture/scalar arrays for one chunk of 128
    face slots (index -1 = inert dummy)."""
    f = np.asarray(faces)
    dummy = f < 0
    fi = np.where(dummy, 0, f)

    def D(a):  # zero out dummies
        a = np.asarray(a, np.float64).copy()
        a[dummy] = 0.0
        return a

    # quantity order: U01,LD01,U12,LD12,U20,LD20,W0,W1,W2 -> coef[4, 9, 128]
    coef = np.zeros((4, 9, 128))
    for e in range(3):
        coef[:, 2 * e, :] = D(prep['UT'][e][fi]).T
        coef[:, 2 * e + 1, :] = D(prep['S2'][e][fi]).T
    coef[:, 6, :] = D(prep['W0c'][fi]).T
    coef[:, 7, :] = D(prep['W1c'][fi]).T
    # dummies: W0=W1=-1 (outside, wc2=1), LD=10 (dist 10 -> prob 0),
    # iz=0.011 -> zp~90.9 -> zn~0.092 (never the argmax), halfL=0.5
    coef[0, 1, dummy] = 10.0
    coef[0, 3, dummy] = 10.0
    coef[0, 5, dummy] = 10.0
    coef[0, 6, dummy] = -1.0
    coef[0, 7, dummy] = -1.0
    coef[:, 8, :] = -coef[:, 6, :] - coef[:, 7, :]
    coef[0, 8, :] += 1.0                                   # w2 = 1 - w0 - w1
    cs = _split3(coef)
    coefk = np.zeros((NK, 9, 128), np.float32)
    for t, (ci, bi) in enumerate(COMBOS):
        coefk[4 * t:4 * t + 4] = cs[ci].astype(np.float32)

    tex = np.asarray(textures, np.float64)[0][fi]          # [128,3,3] (k,c)
    tex[dummy] = 0.0

    scal = np.zeros((128, 9))
    izf = prep['iz'][fi]
    izf[dummy] = 0.011
    scal[:, 0:3] = izf
    hlf = prep['HL'][:, fi].T
    hlf[dummy] = 0.5
    scal[:, 3:6] = hlf
    scal[:, 6:9] = -hlf
    return coefk, tex, scal


def _build_program(pattern, mhat):
    """One SPMD Bass program; chunk counts per position given by pattern."""
    totc = sum(pattern)
    kmax = max(pattern)
    nc = bacc.Bacc("TRN2", target_bir_lowering=False, debug=False,
                   num_devices=NCORES)
    d_coef = nc.dram_tensor("coef", [totc, 24, 9 * 128], F32R, kind="ExternalInput")
    d_basis = nc.dram_tensor("basis", [NPOS, 24, TP], F32R, kind="ExternalInput")
    d_tex = nc.dram_tensor("tex", [128, totc * 9], FP, kind="ExternalInput")
    d_scal = nc.dram_tensor("scal", [128, totc * 9], FP, kind="ExternalInput")
    d_out = nc.dram_tensor("out6", [5, NPOS * TP], FP, kind="ExternalOutput")

    with ExitStack() as ctx:
        tc = ctx.enter_context(tile.TileContext(nc))
        const = ctx.enter_context(tc.tile_pool(name="const", bufs=1))
        stage = ctx.enter_context(tc.tile_pool(name="stage", bufs=3))
        basp = ctx.enter_context(tc.tile_pool(name="basp", bufs=3))
        work = ctx.enter_context(tc.tile_pool(name="work", bufs=2))
        store = ctx.enter_context(tc.tile_pool(name="store", bufs=3))
        zm = ctx.enter_context(tc.tile_pool(name="zm", bufs=3))
        qp = ctx.enter_context(tc.tile_pool(name="qp", bufs=6, space="PSUM"))
        accp = ctx.enter_context(tc.tile_pool(name="accp", bufs=2, space="PSUM"))

        onesc = const.tile([128, 1], BF)
        nc.vector.memset(onesc, 1.0)
        onesf = const.tile([128, 1], FP)
        nc.vector.memset(onesf, 1.0)
        b_sqrt = const.tile([128, 1], FP)
        nc.vector.memset(b_sqrt, 1e-12)
        b_ln = const.tile([128, 1], FP)
        nc.vector.memset(b_ln, 1e-30)
        b_exp = const.tile([128, 1], FP)
        nc.vector.memset(b_exp, -mhat / GAMMA)
        tex_sb = const.tile([128, totc * 9], FP)
        nc.sync.dma_start(out=tex_sb, in_=d_tex[:, :])
        scal_sb = const.tile([128, totc * 9], FP)
        nc.sync.dma_start(out=scal_sb, in_=d_scal[:, :])

        jj = 0
        base = [0]
        for p in pattern:
            base.append(base[-1] + p)
        for pp in range(0, NPOS, 2):
            pair = [pp, pp + 1]
            st8 = {}
            for pos in pair:
                K = pattern[pos]
                bas = basp.tile([24, TP], F32R, tag="bas")
                nc.sync.dma_start(out=bas, in_=d_basis[pos, :, :])

                zn_st = store.tile([128, kmax, TP], FP, tag="zn_st")
                pv_st = store.tile([128, kmax, TP], FP, tag="pv_st")
                wn_st = [store.tile([128, kmax, TP], FP, tag=f"wn{k}_st",
                                     name=f"wn{k}_st") for k in range(3)]
                d2_st = store.tile([128, kmax, TP], FP, tag="d2_st")
                sb_st = store.tile([128, kmax, TP], mybir.dt.uint32,
                                   tag="sb_st")
                acc = accp.tile([65, TP], FP, tag="acc")
                st8[pos] = (zn_st, pv_st, wn_st, d2_st, sb_st, acc)

                # -- phase 1a: per chunk, everything except sqrt/sig/ln/exp --
                for j in range(K):
                    cj = base[pos] + j
                    st = stage.tile([24, 9 * 128], F32R, tag="st")
                    nc.sync.dma_start(out=st, in_=d_coef[cj, :, :])
                    q = [qp.tile([128, TP], FP, tag="q", name=f"q{qi}")
                         for qi in range(9)]
                    for qi in range(9):
                        nc.tensor.matmul(q[qi],
                                         st[:, qi * 128:(qi + 1) * 128],
                                         bas, start=True, stop=True)
                    sc = lambda i: scal_sb[:, cj * 9 + i: cj * 9 + i + 1]

                    # d2_e = LD^2 + relu(|U - L/2| - L/2)^2 (U-L/2 from PE)
                    ru = [work.tile([128, TP], FP, tag=f"ru{e}",
                                    name=f"ru{e}") for e in range(3)]
                    tt = [work.tile([128, TP], FP, tag=f"t{e}",
                                    name=f"t{e}") for e in range(3)]
                    for e in range(3):
                        nc.scalar.activation(ru[e], q[2 * e + 1], AF.Square)
                        nc.vector.tensor_scalar(
                            out=tt[e].bitcast(mybir.dt.uint32),
                            in0=q[2 * e].bitcast(mybir.dt.uint32),
                            scalar1=0x7fffffff, scalar2=None,
                            op0=AL.bitwise_and)                 # |U - L/2|
                        nc.scalar.activation(tt[e], tt[e], AF.Relu,
                                             bias=sc(6 + e))    # overshoot
                        nc.scalar.activation(tt[e], tt[e], AF.Square)
                        nc.vector.tensor_tensor(out=ru[e], in0=ru[e],
                                                in1=tt[e], op=AL.add)
                    nc.vector.tensor_tensor(out=ru[0], in0=ru[0], in1=ru[1],
                                            op=AL.min)
                    nc.vector.tensor_tensor(out=d2_st[:, j, :], in0=ru[0],
                                            in1=ru[2], op=AL.min)

                    cw0 = work.tile([128, TP], FP, tag="cw0")
                    nc.scalar.activation(cw0, q[6], AF.Copy)
                    cw1 = work.tile([128, TP], FP, tag="cw1")
                    nc.scalar.activation(cw1, q[7], AF.Copy)
                    cw2 = work.tile([128, TP], FP, tag="cw2")
                    nc.scalar.activation(cw2, q[8], AF.Copy)
                    m1 = work.tile([128, TP], FP, tag="m1")
                    nc.vector.tensor_tensor(out=m1, in0=cw0, in1=cw1,
                                            op=AL.min)
                    nc.vector.tensor_tensor(out=m1, in0=m1, in1=cw2,
                                            op=AL.min)
                    nc.vector.tensor_scalar(out=sb_st[:, j, :],
                                            in0=m1.bitcast(mybir.dt.uint32),
                                            scalar1=0x80000000, scalar2=None,
                                            op0=AL.bitwise_and)

                    wc0, wc1, wc2 = cw0, cw1, cw2
                    nc.vector.tensor_scalar(out=wc0, in0=cw0, scalar1=0.0,
                                            scalar2=1.0, op0=AL.max,
                                            op1=AL.min)
                    nc.vector.tensor_scalar(out=wc1, in0=cw1, scalar1=0.0,
                                            scalar2=1.0, op0=AL.max,
                                            op1=AL.min)
                    nc.vector.tensor_scalar(out=wc2, in0=cw2, scalar1=0.0,
                                            scalar2=1.0, op0=AL.max,
                                            op1=AL.min)
                    s01 = work.tile([128, TP], FP, tag="s01")
                    nc.vector.tensor_tensor(out=s01, in0=wc0, in1=wc1,
                                            op=AL.add)
                    nc.vector.tensor_tensor(out=s01, in0=s01, in1=wc2,
                                            op=AL.add)
                    invs = work.tile([128, TP], FP, tag="invs")
                    nc.vector.reciprocal_approx_fast(out=invs, in_=s01)
                    r1 = work.tile([128, TP], FP, tag="r1")
                    nc.vector.tensor_scalar(out=r1, in0=wc0, scalar1=sc(0),
                                            scalar2=None, op0=AL.mult)
                    nc.vector.scalar_tensor_tensor(out=r1, in0=wc1,
                                                   scalar=sc(1), in1=r1,
                                                   op0=AL.mult, op1=AL.add)
                    nc.vector.scalar_tensor_tensor(out=r1, in0=wc2,
                                                   scalar=sc(2), in1=r1,
                                                   op0=AL.mult, op1=AL.add)
                    nc.vector.tensor_tensor(out=r1, in0=r1, in1=invs,
                                            op=AL.mult)
                    nc.vector.reciprocal_approx_fast(out=s01, in_=r1)  # zp
                    nc.vector.tensor_scalar(out=zn_st[:, j, :], in0=s01,
                                            scalar1=-1.0 / (FAR - NEAR),
                                            scalar2=FAR / (FAR - NEAR),
                                            op0=AL.mult, op1=AL.add)
                    for k, wck in enumerate([wc0, wc1, wc2]):
                        nc.gpsimd.tensor_tensor(out=wn_st[k][:, j, :],
                                                in0=wck, in1=invs,
                                                op=AL.mult)

            # -- phase 1b over the PAIR: one table load per LUT fn serves 2
            for pos in pair:
                zn_st, pv_st, wn_st, d2_st, sb_st, acc = st8[pos]
                for j in range(pattern[pos]):
                    nc.scalar.activation(d2_st[:, j, :], d2_st[:, j, :],
                                         AF.Sqrt, bias=b_sqrt)
            for pos in pair:
                zn_st, pv_st, wn_st, d2_st, sb_st, acc = st8[pos]
                for j in range(pattern[pos]):
                    nc.vector.tensor_tensor(
                        out=d2_st[:, j, :].bitcast(mybir.dt.uint32),
                        in0=d2_st[:, j, :].bitcast(mybir.dt.uint32),
                        in1=sb_st[:, j, :], op=AL.bitwise_or)
            for pos in pair:
                zn_st, pv_st, wn_st, d2_st, sb_st, acc = st8[pos]
                for j in range(pattern[pos]):
                    nc.scalar.activation(pv_st[:, j, :], d2_st[:, j, :],
                                         AF.Sigmoid, scale=1.0 / SIGMA)
            for pos in pair:
                zn_st, pv_st, wn_st, d2_st, sb_st, acc = st8[pos]
                for j in range(pattern[pos]):
                    q1m = work.tile([128, TP], FP, tag="q1m")
                    nc.vector.tensor_scalar(out=q1m, in0=pv_st[:, j, :],
                                            scalar1=-1.0, scalar2=1.0,
                                            op0=AL.mult, op1=AL.add)
                    lq = work.tile([128, TP], FP, tag="lq")
                    nc.scalar.activation(lq, q1m, AF.Ln, bias=b_ln)
                    nc.tensor.matmul(acc[64:65, :], onesf[:, 0:1], lq,
                                     start=(j == 0),
                                     stop=(j == pattern[pos] - 1))

            # -- phase 3 over the pair: exp weights, rgb/dsum accumulation --
            for pos in pair:
                zn_st, pv_st, wn_st, d2_st, sb_st, acc = st8[pos]
                K = pattern[pos]
                for j in range(K):
                    cj = base[pos] + j
                    d = work.tile([128, TP], FP, tag="d")
                    nc.scalar.activation(d, zn_st[:, j, :], AF.Exp,
                                         scale=1.0 / GAMMA, bias=b_exp)
                    nc.vector.tensor_tensor(out=d, in0=pv_st[:, j, :],
                                            in1=d, op=AL.mult)     # wexp
                    for k in range(3):
                        g = work.tile([128, TP], FP, tag="g", bufs=3)
                        nc.gpsimd.tensor_tensor(out=g, in0=d,
                                                in1=wn_st[k][:, j, :],
                                                op=AL.mult)
                        nc.tensor.matmul(
                            acc[0:3, :],
                            tex_sb[:, cj * 9 + k * 3: cj * 9 + (k + 1) * 3],
                            g, start=(j == 0 and k == 0),
                            stop=(j == K - 1 and k == 2))
                    nc.tensor.matmul(acc[32:33, :], onesf[:, 0:1], d,
                                     start=(j == 0), stop=(j == K - 1))

                o6 = zm.tile([65, TP], FP, tag="o6")
                nc.vector.tensor_copy(o6[0:3, :], acc[0:3, :])
                nc.vector.tensor_copy(o6[32:33, :], acc[32:33, :])
                nc.scalar.activation(o6[64:65, :], acc[64:65, :], AF.Copy)
                nc.sync.dma_start(out=d_out[0:3, pos * TP:(pos + 1) * TP],
                                  in_=o6[0:3, :])
                nc.sync.dma_start(out=d_out[3:4, pos * TP:(pos + 1) * TP],
                                  in_=o6[32:33, :])
                nc.sync.dma_start(out=d_out[4:5, pos * TP:(pos + 1) * TP],
                                  in_=o6[64:65, :])
    nc.compile()
    return nc


def kernel(face_vertices, face_textures):
    prep = _host_prep(face_vertices)
    pattern, assign, mhat = _cull_and_balance(prep)
    totc = sum(pattern)

    pix = ((np.arange(H, dtype=np.float64) + 0.5) / H) * 2.0 - 1.0
    in_maps = []
    for c in range(NCORES):
        coef = np.zeros((totc, NK, 9 * 128), np.float32)
        tex = np.zeros((128, totc * 9), np.float32)
        scal = np.zeros((128, totc * 9), np.float32)
        basis = np.zeros((NPOS, NK, TP), np.float32)
        jj = 0
        for pos in range(NPOS):
            b, faces = assign[c][pos]
            yb, xb = b // 2, b % 2
            py = np.repeat(pix[4 * yb:4 * yb + 4], 128)
            px = np.tile(pix[128 * xb:128 * xb + 128], 4)
            b4 = np.stack([np.ones(TP), px, py, px ** 2 + py ** 2])
            bs = _split3(b4)
            for t, (ci, bi) in enumerate(COMBOS):
                basis[pos, 4 * t:4 * t + 4] = bs[bi].astype(np.float32)
            for j in range(pattern[pos]):
                cf, tx, sl = _face_arrays(prep, face_textures,
                                          faces[j * 128:(j + 1) * 128])
                coef[jj] = cf.reshape(NK, 9 * 128)
                tex[:, jj * 9:(jj + 1) * 9] = tx.reshape(128, 9)
                scal[:, jj * 9:(jj + 1) * 9] = sl
                jj += 1
        in_maps.append({"coef": coef, "basis": basis, "tex": tex, "scal": scal})

    nc = _build_program(pattern, mhat)
    global LAST_RESULT
    if TRACE:
        res = run_bass_kernel_spmd(nc, in_maps, core_ids=list(range(NCORES)),
                                   trace=True)
    else:
        res = run_bass_kernel_spmd(nc, in_maps, core_ids=list(range(NCORES)))
    LAST_RESULT = res

    out = np.zeros((1, 4, H, W), np.float32)
    for c in range(NCORES):
        o6 = res.results[c]["out6"]                        # [6, NPOS*TP]
        for pos in range(NPOS):
            b, _ = assign[c][pos]
            yb, xb = b // 2, b % 2
            seg = o6[:, pos * TP:(pos + 1) * TP]
            wbg = np.float32(np.exp((EPS - mhat) / GAMMA))
            dsum = seg[3] + wbg
            rgb = seg[0:3] / dsum[None]
            alpha = 1.0 - np.exp(seg[4])
            ys = slice(4 * yb, 4 * yb + 4)
            xs = slice(128 * xb, 128 * xb + 128)
            out[0, 0:3, ys, xs] = rgb.reshape(3, 4, 128)
            out[0, 3, ys, xs] = alpha.reshape(4, 128)
    return out
